# revision 17
# baseline (speedup 1.0000x reference)
"""Trainium2 Bass kernel for a dense transformer block (LN->causal attn->res->LN->MLP->res).

Shapes (hardcoded): x [2, 2048, 1024], 16 heads, head_dim 64, MLP hidden 4096, fp32 out.

v6 sharding: 8 cores = (batch b in {0,1}) x (token class j in {0..3}).
Class j = tokens {t : t % 4 == j} (512 tokens, position order).  Each core
gets the full 2048-token context CLASS-MAJOR with its own class LAST
(block order (j+1)%4, (j+2)%4, (j+3)%4, j), computes LN1 + K/V over the whole
context and Q for its own class, then runs causal-skip attention: any class
block's 128-token tile t covers positions [512t, 512(t+1)), so key tile
(block, t) is needed only by query tiles q >= t -- tiles with t > q are never
computed (37.5% of score/exp/AV work skipped, uniformly on every core; the
interleave balances the causal triangle).  The ragged diagonal (q == t) is a
per-core 0/1 mask (tri if the block's class <= own class else strictly-lower)
multiplied after the exp.  LN2/MLP/residuals for the own 512 tokens; the host
scatters rows back to positions j::4.  (A K/V AllGather variant was tried:
the 4-core gather costs ~120us/collective on this runtime and loses to the
~80us of duplicated K/V compute it saves.)

P1/P2 are fused into one batch pipeline: per 4-tile x batch, LN -> (Q for the
own batch) -> the K context chunk and V tiles that batch enables, so the PE
never waits for the whole LN pass (engines execute in program order; the
earlier phase-ordered version left the PE stalled behind not-yet-ready
transposes).

Carried over from v3/v4: ONE packed uint8 input tensor (runtime staging costs
~63us per tensor + ~10us/MB per exec, dominating the wall clock); bf16
weights (fp8 fails the 2e-2 error gate); bf16-only x; V augmented with a
per-head ones column accumulating the softmax denominator.
"""

from contextlib import ExitStack

import numpy as np

import concourse.bacc as bacc
import concourse.mybir as mybir
import concourse.tile as tile
from concourse.masks import make_identity

F32 = mybir.dt.float32
BF16 = mybir.dt.bfloat16
FP8 = mybir.dt.float8e4
AF = mybir.ActivationFunctionType
ALU = mybir.AluOpType

B = 2
T = 2048
D = 1024
H = 16
HD = 64
HDA = HD + 1  # +1 denominator column per head
MLP = 4096
NQ = 512  # tokens per core
CTX = T
EPS = 1e-5

N_CORES = 8
P = 128
CLS = 4

D_T = D // P  # 8
Q_T = NQ // P  # 4 query tiles (also key tiles per class)
M_T = MLP // P  # 32
VA = H * HDA  # 1040 augmented V width

REPLICA_GROUPS = [[0, 1, 2, 3], [4, 5, 6, 7]]

# ---- packed-input layout (bytes). All segments 4KB-aligned. ----
def _align(x, a=4096):
    return (x + a - 1) // a * a


_off = 0
def _seg(nbytes):
    global _off
    o = _off
    _off = _align(_off + nbytes)
    return o


OFF_X = _seg(CTX * D * 2)           # bf16 [2048,1024] class-major, own class last
OFF_WQ = _seg(D * D * 2)            # bf16 [1024,1024] pretiled
OFF_WK = _seg(D * D * 2)            # bf16 [1024,1024] pretiled
OFF_WVA = _seg(D * VA * 2)          # bf16 [1024,1040]
OFF_WFC = _seg(MLP * D * 2)         # bf16 [4096,1024] pretiled
OFF_WPJ = _seg(D * MLP * 2)         # bf16 [1024,4096] pretiled
OFF_TRI = _seg(P * CLS * 2 * P)     # fp8 0/1 [128, 4, 2, 128]
OFF_BQK = _seg(P * 2 * D_T * 4)     # f32 [128,16]
OFF_BFC = _seg(P * M_T * 4)         # f32 [128,32]
OFF_BPJ = _seg(P * D_T * 4)         # f32 [128,8]
OFF_BVA = _seg(VA * 2)              # bf16 [1,1040]
NB = _align(_off)


def build_program(loop_n: int = 1, bv_nonzero: bool = False):
    """Emit the SPMD Bass program. Returns finalized nc."""
    nc = bacc.Bacc("TRN2", target_bir_lowering=False)

    pk = nc.dram_tensor("pk", [1, NB], mybir.dt.uint8, kind="ExternalInput")
    out = nc.dram_tensor("out", [NQ, D], F32, kind="ExternalOutput")

    def view(off, nbytes, dt):
        return pk[0, off : off + nbytes].bitcast(dt)

    with tile.TileContext(nc) as tc:
        with ExitStack() as ctx:
            if loop_n > 1:
                ctx.enter_context(tc.For_i(0, loop_n, 1))
            const = ctx.enter_context(tc.tile_pool(name="const", bufs=1))
            identity = const.tile([P, P], F32)
            make_identity(nc, identity)
            identity_bf = const.tile([P, P], BF16)
            make_identity(nc, identity_bf)
            ones1 = const.tile([1, P], BF16)
            nc.vector.memset(ones1, 1.0)
            eps_t = const.tile([P, 1], F32)
            nc.vector.memset(eps_t, EPS)
            bqk_sb = const.tile([P, 2 * D_T], F32)
            nc.sync.dma_start(
                bqk_sb, view(OFF_BQK, P * 2 * D_T * 4, F32).rearrange("(p c) -> p c", p=P)
            )
            bva_sb = const.tile([1, VA], BF16)
            nc.sync.dma_start(
                bva_sb, view(OFF_BVA, VA * 2, BF16).rearrange("(p c) -> p c", p=1)
            )
            tri8 = const.tile([P, CLS, 2, P], FP8)
            tri_sb = const.tile([P, CLS, 2, P], BF16)

            # Long-lived pools.
            qt_cm = tc.tile_pool(name="qt", bufs=1)
            qt_pool = qt_cm.__enter__()
            QT = [qt_pool.tile([P, NQ], BF16, name=f"QT{i}") for i in range(D_T)]
            kt_cm = tc.tile_pool(name="ktp", bufs=1)
            kt_pool = kt_cm.__enter__()
            KTT = kt_pool.tile([P, D_T, CTX], BF16, name="KTT")
            vsb_cm = tc.tile_pool(name="vsb", bufs=1)
            vsb_pool = vsb_cm.__enter__()
            VSBT = vsb_pool.tile([P, CLS * Q_T, VA], BF16, name="VSBT")

            # RIGHT pools (live into P4/P5)
            yt_pool = ctx.enter_context(tc.tile_pool(name="yt", bufs=1, side="right"))
            YT = [yt_pool.tile([P, NQ], F32, name=f"YT{i}") for i in range(D_T)]
            x2_pool = ctx.enter_context(tc.tile_pool(name="x2", bufs=1, side="right"))
            X2 = [x2_pool.tile([P, D], F32, name=f"X2{i}") for i in range(Q_T)]
            l2t_pool = ctx.enter_context(
                tc.tile_pool(name="l2t", bufs=1, side="right")
            )
            L2T = [l2t_pool.tile([P, NQ], BF16, name=f"L2T{i}") for i in range(D_T)]
            xo_pool = ctx.enter_context(tc.tile_pool(name="xo", bufs=1, side="right"))
            XO = xo_pool.tile([P, Q_T, D], BF16, name="XO")
            # xnT on top of the RIGHT stack; freed after Q/K/V, wfc prefetch
            # reuses the space during attention.
            xnt_cm = tc.tile_pool(name="xnt", bufs=1, side="right")
            xnt_pool = xnt_cm.__enter__()
            xnT = [xnt_pool.tile([P, CTX], BF16, name=f"xnT{i}") for i in range(D_T)]

            # -------- P1+P2 fused: per x-batch LN -> (Q) -> K chunk -> V tiles
            # PE stays fed: batch 0 (own class, ctx cols 1536:2048) lands
            # first, then Q, then each later batch's LN overlaps the previous
            # batch's K/V matmuls.  wk streams per (nt, mt); WVA is resident.
            xcb_v = view(OFF_X, CTX * D * 2, BF16)
            batches = ((12, 13, 14, 15), (0, 1, 2, 3), (4, 5, 6, 7), (8, 9, 10, 11))
            ntof = (3, 0, 1, 2)  # K context chunk produced after each batch
            with tc.tile_pool(name="p2vw", bufs=1) as p2vw, tc.tile_pool(
                name="p1work", bufs=2
            ) as p1w, tc.tile_pool(name="p1xn", bufs=4) as p1xn, tc.tile_pool(
                name="p1stat", bufs=6
            ) as p1s, tc.tile_pool(name="p2w", bufs=2) as p2w, tc.tile_pool(
                name="p1ps", bufs=4, space="PSUM"
            ) as p1ps, tc.tile_pool(
                name="p2ps", bufs=2, space="PSUM"
            ) as p2ps, tc.tile_pool(
                name="p2vps", bufs=2, space="PSUM", side="right"
            ) as p2vps:
                WVA = p2vw.tile([P, D_T, VA], BF16, name="wva")
                vchunks = [(0, 512), (512, 512), (1024, VA - 1024)]
                for bi, bt in enumerate(batches):
                    if bi == 0:
                        xt = XO
                    else:
                        xt = p1w.tile([P, 4, D], BF16, tag="xt")
                    nc.sync.dma_start(
                        xt,
                        xcb_v[bt[0] * P * D : (bt[0] + 4) * P * D].rearrange(
                            "(a p c) -> p a c", p=P, c=D
                        ),
                    )
                    if bi == 0:
                        nc.sync.dma_start(
                            WVA,
                            view(OFF_WVA, D * VA * 2, BF16).rearrange(
                                "(a p c) -> p a c", p=P, c=VA
                            ),
                        )
                    xns = []
                    for ai, tt in enumerate(bt):
                        stats = p1s.tile([P, 2, 6], F32, tag="stats")
                        for g in range(2):
                            nc.vector.bn_stats(
                                stats[:, g, :], xt[:, ai, g * 512 : (g + 1) * 512]
                            )
                        mv = p1s.tile([P, 2], F32, tag="mv")
                        nc.vector.bn_aggr(mv, stats)
                        sd = p1s.tile([P, 1], F32, tag="sd")
                        nc.scalar.activation(sd, mv[:, 1:2], AF.Sqrt, bias=eps_t)
                        rstd = p1s.tile([P, 1], F32, tag="rstd")
                        nc.vector.reciprocal(rstd, sd)
                        nmb = p1s.tile([P, 1], F32, tag="nmb")
                        nc.vector.tensor_scalar(
                            nmb, mv[:, 0:1], rstd, -1.0, ALU.mult, ALU.mult
                        )
                        xn = p1xn.tile([P, D], BF16, tag="xn")
                        nc.scalar.activation(
                            xn, xt[:, ai, :], AF.Identity, bias=nmb, scale=rstd
                        )
                        xns.append(xn)
                    for dt_ in range(D_T):
                        tp = p1ps.tile([P, 4, P], BF16, tag="tp")
                        for ai in range(4):
                            nc.tensor.transpose(
                                tp[:, ai, :],
                                xns[ai][:, dt_ * P : (dt_ + 1) * P],
                                identity_bf,
                            )
                        if dt_ % 2 == 0:
                            nc.vector.tensor_copy(
                                xnT[dt_][:, bt[0] * P : (bt[0] + 4) * P], tp
                            )
                        else:
                            nc.scalar.copy(
                                xnT[dt_][:, bt[0] * P : (bt[0] + 4) * P], tp
                            )
                    nt = ntof[bi]
                    if bi == 0:
                        # Q^T for the own class (ctx cols 1536:2048)
                        for mt in range(D_T):
                            ws = p2w.tile([P, D_T, P], BF16, tag="wsk")
                            nc.sync.dma_start(
                                ws,
                                view(
                                    OFF_WQ + mt * P * D * 2, P * D * 2, BF16
                                ).rearrange("(p a c) -> p a c", p=P, c=P),
                            )
                            ps = p2ps.tile([P, NQ], F32, tag="ps")
                            for kt_ in range(D_T):
                                nc.tensor.matmul(
                                    ps,
                                    ws[:, kt_, :],
                                    xnT[kt_][:, CTX - NQ :],
                                    start=(kt_ == 0),
                                    stop=(kt_ == D_T - 1),
                                )
                            nc.scalar.activation(
                                QT[mt], ps, AF.Identity, bias=bqk_sb[:, mt : mt + 1]
                            )
                    # K^T chunk nt (ctx cols nt*512 .. +512)
                    for mt in range(D_T):
                        ws = p2w.tile([P, D_T, P], BF16, tag="wsk")
                        nc.sync.dma_start(
                            ws,
                            view(OFF_WK + mt * P * D * 2, P * D * 2, BF16).rearrange(
                                "(p a c) -> p a c", p=P, c=P
                            ),
                        )
                        ps = p2ps.tile([P, 512], F32, tag="ps")
                        for kt_ in range(D_T):
                            nc.tensor.matmul(
                                ps,
                                ws[:, kt_, :],
                                xnT[kt_][:, nt * 512 : (nt + 1) * 512],
                                start=(kt_ == 0),
                                stop=(kt_ == D_T - 1),
                            )
                        nc.vector.tensor_scalar_add(
                            KTT[:, mt, nt * 512 : (nt + 1) * 512],
                            ps,
                            bqk_sb[:, D_T + mt : D_T + mt + 1],
                        )
                    # V_aug for this batch's 4 context tiles
                    for mtv in bt:
                        for ci, (c0, cw) in enumerate(vchunks):
                            ps = p2vps.tile([P, 512], F32, tag="ps")
                            for kt_ in range(D_T):
                                nc.tensor.matmul(
                                    ps[:, :cw],
                                    xnT[kt_][:, mtv * P : (mtv + 1) * P],
                                    WVA[:, kt_, c0 : c0 + cw],
                                    start=(kt_ == 0),
                                    stop=(kt_ == D_T - 1 and not bv_nonzero),
                                )
                            if bv_nonzero:
                                nc.tensor.matmul(
                                    ps[:, :cw],
                                    ones1,
                                    bva_sb[:, c0 : c0 + cw],
                                    start=False,
                                    stop=True,
                                )
                            if ci % 2 == 0:
                                nc.vector.tensor_copy(
                                    VSBT[:, mtv, c0 : c0 + cw], ps[:, :cw]
                                )
                            else:
                                nc.scalar.copy(
                                    VSBT[:, mtv, c0 : c0 + cw], ps[:, :cw]
                                )
                if not bv_nonzero:
                    ones_cols = VSBT.rearrange("p t (h c) -> p t h c", c=HDA)[
                        :, :, :, HD : HD + 1
                    ]
                    nc.vector.memset(ones_cols, 1.0)

            # xnT consumed -> free; prefetch half of wfc during attention.
            xnt_cm.__exit__(None, None, None)
            MT_RES = M_T // 2
            wfc_cm = tc.tile_pool(name="wfcp_sb", bufs=1, side="right")
            wfc_pool = wfc_cm.__enter__()
            WFC = wfc_pool.tile([P, MT_RES, D_T, P], BF16, name="WFC")
            nc.sync.dma_start(
                WFC,
                view(OFF_WFC, MT_RES * P * D * 2, BF16).rearrange(
                    "(a p c) -> p a c", p=P, c=D
                ).rearrange("p a (k c) -> p a k c", c=P),
            )

            # -------- P3: causal-skip attention, key tile (c, t) -------------
            # key tile (class c, tile t) serves query tiles q in [t, 4); the
            # first 128 query columns (q == t) get the ragged tri mask.
            nc.sync.dma_start(
                tri8,
                view(OFF_TRI, P * CLS * 2 * P, FP8).rearrange(
                    "(p c s q) -> p c s q", p=P, c=CLS, s=2
                ),
            )
            nc.vector.tensor_copy(tri_sb, tri8)
            ptp_cm = tc.tile_pool(name="ptp", bufs=4)
            ptp = ptp_cm.__enter__()
            p3s_cm = tc.tile_pool(name="p3s", bufs=2)
            p3s = p3s_cm.__enter__()
            stps_cm = tc.tile_pool(name="stps", bufs=2, space="PSUM")
            stps = stps_cm.__enter__()
            yps_cm = tc.tile_pool(name="yps", bufs=2, space="PSUM")
            yps = yps_cm.__enter__()
            for hp in range(H // 2):
                yp = yps.tile([HDA, 2, NQ], F32, name=f"yp{hp}", tag="yp")
                for t in range(Q_T):
                    nqc = (Q_T - t) * P  # query columns t*128 .. 512
                    for c in range(CLS):
                        kti = c * Q_T + t
                        kcol = c * NQ + t * P
                        # fixed 512-wide halves keep each matmul's PSUM
                        # region inside one 2KB bank
                        st = stps.tile([P, 2, NQ], F32, tag="st")
                        for s in range(2):
                            nc.tensor.matmul(
                                st[:, s, :nqc],
                                KTT[s * HD : (s + 1) * HD, hp, kcol : kcol + P],
                                QT[hp][s * HD : (s + 1) * HD, t * P :],
                                start=True,
                                stop=True,
                                tile_position=(s * HD, 0),
                            )
                        pt = ptp.tile([P, 2, nqc], BF16, tag="pt")
                        nc.scalar.activation(pt, st[:, :, :nqc], AF.Exp)
                        nc.vector.tensor_mul(
                            pt[:, :, 0:P], pt[:, :, 0:P], tri_sb[:, c, :, :]
                        )
                        # start=True zeroes the whole 2KB PSUM bank (one
                        # bank per s), so later sub-range accumulations are
                        # against zeroed/accumulated state; one start at
                        # (t=0,c=0), one stop at (t=3,c=3) per bank.
                        for s in range(2):
                            h = 2 * hp + s
                            nc.tensor.matmul(
                                yp[:, s, t * P :],
                                VSBT[:, kti, h * HDA : (h + 1) * HDA],
                                pt[:, s, :],
                                start=(t == 0 and c == 0),
                                stop=(t == Q_T - 1 and c == CLS - 1),
                            )
                for s in range(2):
                    ysb = p3s.tile([HDA, NQ], F32, name=f"ysb{hp}_{s}", tag="ysb")
                    if s == 0:
                        nc.vector.tensor_copy(ysb, yp[:, s, :])
                    else:
                        nc.scalar.copy(ysb, yp[:, s, :])
                    recip = p3s.tile([1, NQ], F32, tag="recip")
                    nc.vector.reciprocal(recip, ysb[HD : HD + 1, :])
                    rb = p3s.tile([HD, NQ], F32, tag="rb")
                    nc.gpsimd.partition_broadcast(rb, recip)
                    nc.vector.tensor_mul(
                        YT[hp][s * HD : (s + 1) * HD, :], ysb[:HD, :], rb
                    )

            yps_cm.__exit__(None, None, None)
            stps_cm.__exit__(None, None, None)
            p3s_cm.__exit__(None, None, None)
            ptp_cm.__exit__(None, None, None)
            vsb_cm.__exit__(None, None, None)
            kt_cm.__exit__(None, None, None)
            qt_cm.__exit__(None, None, None)

            # ---------------- P4: residual + LN2 + transpose -----------------
            with tc.tile_pool(name="p4w", bufs=3) as p4w, tc.tile_pool(
                name="p4s", bufs=4
            ) as p4s, tc.tile_pool(name="p4ps", bufs=4, space="PSUM") as p4ps:
                for tt in range(Q_T):
                    for mt in range(D_T):
                        tp = p4ps.tile([P, P], F32, tag="tp")
                        nc.tensor.transpose(
                            tp, YT[mt][:, tt * P : (tt + 1) * P], identity
                        )
                        nc.vector.tensor_add(
                            X2[tt][:, mt * P : (mt + 1) * P],
                            XO[:, tt, mt * P : (mt + 1) * P],
                            tp,
                        )
                    stats = p4s.tile([P, 2, 6], F32, tag="stats2")
                    for g in range(2):
                        nc.vector.bn_stats(
                            stats[:, g, :], X2[tt][:, g * 512 : (g + 1) * 512]
                        )
                    mv = p4s.tile([P, 2], F32, tag="mv2")
                    nc.vector.bn_aggr(mv, stats)
                    sd = p4s.tile([P, 1], F32, tag="sd2")
                    nc.scalar.activation(sd, mv[:, 1:2], AF.Sqrt, bias=eps_t)
                    rstd = p4s.tile([P, 1], F32, tag="rstd2")
                    nc.vector.reciprocal(rstd, sd)
                    nmb = p4s.tile([P, 1], F32, tag="nmb2")
                    nc.vector.tensor_scalar(
                        nmb, mv[:, 0:1], rstd, -1.0, ALU.mult, ALU.mult
                    )
                    l2 = p4w.tile([P, D], BF16, tag="l2")
                    nc.scalar.activation(l2, X2[tt], AF.Identity, bias=nmb, scale=rstd)
                    for mt in range(D_T):
                        tp = p4ps.tile([P, P], BF16, tag="tpb")
                        nc.tensor.transpose(
                            tp, l2[:, mt * P : (mt + 1) * P], identity_bf
                        )
                        if mt % 2 == 0:
                            nc.vector.tensor_copy(
                                L2T[mt][:, tt * P : (tt + 1) * P], tp
                            )
                        else:
                            nc.scalar.copy(L2T[mt][:, tt * P : (tt + 1) * P], tp)

            # ---------------- P5: MLP + final residual ----------------
            with tc.tile_pool(name="h1t", bufs=1) as h1t_pool, tc.tile_pool(
                name="p5w", bufs=2
            ) as p5w, tc.tile_pool(name="p5o", bufs=1) as p5o, tc.tile_pool(
                name="p5ps", bufs=3, space="PSUM"
            ) as p5ps, tc.tile_pool(
                name="p5tps", bufs=4, space="PSUM"
            ) as p5tps:
                bfc_sb = p5o.tile([P, M_T], F32)
                nc.sync.dma_start(
                    bfc_sb,
                    view(OFF_BFC, P * M_T * 4, F32).rearrange("(p c) -> p c", p=P),
                )
                bproj_sb = p5o.tile([P, D_T], F32)
                nc.sync.dma_start(
                    bproj_sb,
                    view(OFF_BPJ, P * D_T * 4, F32).rearrange("(p c) -> p c", p=P),
                )
                OUT = p5o.tile([P, Q_T, D], F32, name="OUT")
                H1T = [h1t_pool.tile([P, NQ], BF16, name=f"H1T{i}") for i in range(M_T)]
                for mt in range(M_T):
                    if mt < MT_RES:
                        wfc_t = WFC[:, mt]
                    else:
                        wfc_t = p5w.tile([P, D_T, P], BF16, tag="wsf")
                        nc.sync.dma_start(
                            wfc_t,
                            view(OFF_WFC + mt * P * D * 2, P * D * 2, BF16).rearrange(
                                "(p k c) -> p k c", p=P, c=P
                            ),
                        )
                    ps = p5ps.tile([P, NQ], F32, tag="ps")
                    for kt_ in range(D_T):
                        nc.tensor.matmul(
                            ps,
                            wfc_t[:, kt_, :],
                            L2T[kt_],
                            start=(kt_ == 0),
                            stop=(kt_ == D_T - 1),
                        )
                    nc.scalar.activation(
                        H1T[mt], ps, AF.Relu, bias=bfc_sb[:, mt : mt + 1]
                    )
                wfc_cm.__exit__(None, None, None)
                for mt in range(D_T):
                    ws = p5w.tile([P, M_T, P], BF16, tag="wsp")
                    nc.sync.dma_start(
                        ws,
                        view(OFF_WPJ + mt * P * MLP * 2, P * MLP * 2, BF16).rearrange(
                            "(p a c) -> p a c", p=P, c=P
                        ),
                    )
                    ps = p5ps.tile([P, NQ], F32, tag="ps")
                    for kt_ in range(M_T):
                        nc.tensor.matmul(
                            ps,
                            ws[:, kt_, :],
                            H1T[kt_],
                            start=(kt_ == 0),
                            stop=(kt_ == M_T - 1),
                        )
                    mlpt = p5w.tile([P, NQ], F32, tag="mlpt")
                    nc.vector.tensor_scalar_add(mlpt, ps, bproj_sb[:, mt : mt + 1])
                    for tt in range(Q_T):
                        tp = p5tps.tile([P, P], F32, tag="tp")
                        nc.tensor.transpose(
                            tp, mlpt[:, tt * P : (tt + 1) * P], identity
                        )
                        nc.vector.tensor_add(
                            OUT[:, tt, mt * P : (mt + 1) * P],
                            X2[tt][:, mt * P : (mt + 1) * P],
                            tp,
                        )
                    if mt == D_T // 2 - 1 or mt == D_T - 1:
                        h0 = 0 if mt < D_T // 2 else D // 2
                        nc.sync.dma_start(
                            out.rearrange("(a p) c -> p a c", p=P)[
                                :, :, h0 : h0 + D // 2
                            ],
                            OUT[:, :, h0 : h0 + D // 2],
                        )


    nc.finalize()
    return nc


_PROG = {}


def _get_program(bv_nonzero: bool = False):
    if bv_nonzero not in _PROG:
        _PROG[bv_nonzero] = build_program(bv_nonzero=bv_nonzero)
    return _PROG[bv_nonzero]


def _pretile(w, n_out_tiles, n_k_tiles):
    """[K, N] -> lhsT pre-tiled layout: row (mt*128+p), flat col (kt*128+c)
    holds w[kt*128 + p, mt*128 + c]."""
    K, N = w.shape
    assert K == n_k_tiles * P and N == n_out_tiles * P
    # axes (kt, p, mt, c) -> (mt, p, kt, c)
    return np.ascontiguousarray(
        w.reshape(n_k_tiles, P, n_out_tiles, P)
        .transpose(2, 1, 0, 3)
        .reshape(n_out_tiles * P, n_k_tiles * P)
    )


def make_in_maps(x, ln1_scale, ln1_shift, w_qkv, b_qkv, ln2_scale, ln2_shift,
                 w_fc, b_fc, w_proj, b_proj):
    """Host-side prep: fold LN affine into weights, prescale Q by 1/sqrt(hd),
    augment V with the ones column, pre-tile weights (bf16), build the
    per-core class-interleaved x slice + ragged-diagonal masks, and pack
    everything into one uint8 tensor per core."""
    import ml_dtypes

    bf16 = ml_dtypes.bfloat16
    fp8 = mybir.dt.np(FP8)

    x = np.asarray(x, np.float32)
    ln1_scale = np.asarray(ln1_scale, np.float32)
    ln1_shift = np.asarray(ln1_shift, np.float32)
    w_qkv = np.asarray(w_qkv, np.float32)
    b_qkv = np.asarray(b_qkv, np.float32)
    ln2_scale = np.asarray(ln2_scale, np.float32)
    ln2_shift = np.asarray(ln2_shift, np.float32)
    w_fc = np.asarray(w_fc, np.float32)
    b_fc = np.asarray(b_fc, np.float32)
    w_proj = np.asarray(w_proj, np.float32)
    b_proj = np.asarray(b_proj, np.float32)

    # fold LN1 affine into qkv weights
    w1 = ln1_scale[:, None] * w_qkv  # [D, 3D]
    b1 = b_qkv + ln1_shift @ w_qkv  # [3D]
    sc = 1.0 / np.sqrt(HD)
    wq = w1[:, :D] * sc
    bq = b1[:D] * sc
    wk = w1[:, D : 2 * D]
    bk = b1[D : 2 * D]
    wv = w1[:, 2 * D :]
    bv = b1[2 * D :]

    wqp_h = _pretile(wq, D_T, D_T).astype(bf16)
    wkp_h = _pretile(wk, D_T, D_T).astype(bf16)
    bqk_h = np.ascontiguousarray(
        np.concatenate([bq, bk]).reshape(2 * D_T, P).T
    )  # [128, 16] f32

    wva_h = np.zeros((D, VA), np.float32)
    bva_h = np.zeros((1, VA), np.float32)
    for h in range(H):
        wva_h[:, h * HDA : h * HDA + HD] = wv[:, h * HD : (h + 1) * HD]
        bva_h[0, h * HDA : h * HDA + HD] = bv[h * HD : (h + 1) * HD]
        bva_h[0, h * HDA + HD] = 1.0  # denominator ones column
    wva_h = wva_h.astype(bf16)
    bva_h = bva_h.astype(bf16)

    # fold LN2 affine into fc; pre-tile bf16
    wfc_f = ln2_scale[:, None] * w_fc
    wfcp_h = _pretile(wfc_f, M_T, D_T).astype(bf16)  # [4096, 1024]
    wprojp_h = _pretile(w_proj, D_T, M_T).astype(bf16)  # [1024, 4096]
    bfc_h = np.ascontiguousarray((b_fc + ln2_shift @ w_fc).reshape(M_T, P).T)
    bproj_h = np.ascontiguousarray(b_proj.reshape(D_T, P).T)  # [128, 8]

    def put(buf, off, arr):
        bts = np.ascontiguousarray(arr).view(np.uint8).reshape(-1)
        buf[off : off + bts.size] = bts

    base = np.zeros(NB, np.uint8)
    put(base, OFF_WQ, wqp_h)
    put(base, OFF_WK, wkp_h)
    put(base, OFF_WVA, wva_h)
    put(base, OFF_WFC, wfcp_h)
    put(base, OFF_WPJ, wprojp_h)
    put(base, OFF_BQK, bqk_h)
    put(base, OFF_BFC, bfc_h)
    put(base, OFF_BPJ, bproj_h)
    put(base, OFF_BVA, bva_h)

    # ragged diagonal masks: key index i (partition) vs query index col:
    # keep if i <= col (key class c <= own class j) else i < col.
    ii = np.arange(P)[:, None]
    qq = np.arange(P)[None, :]
    tri_inc = (ii <= qq).astype(np.float32)  # [128,128]
    tri_exc = (ii < qq).astype(np.float32)

    in_maps = []
    for core in range(N_CORES):
        b, j = divmod(core, CLS)
        # class-major context, own class last: block bi holds class (j+1+bi)%4
        blocks = [(j + 1 + bi) % CLS for bi in range(CLS)]
        xperm = np.concatenate([x[b, c::CLS] for c in blocks], axis=0)  # [2048, D]
        tri_h = np.empty((P, CLS, 2, P), np.float32)
        for bi, c in enumerate(blocks):
            m = tri_inc if c <= j else tri_exc
            tri_h[:, bi, 0, :] = m
            tri_h[:, bi, 1, :] = m
        pkc = base.copy()
        put(pkc, OFF_X, np.ascontiguousarray(xperm.astype(bf16)))
        put(pkc, OFF_TRI, tri_h.astype(fp8))
        in_maps.append({"pk": pkc.reshape(1, NB)})
    return in_maps


def assemble_output(results):
    out = np.empty((B, T, D), np.float32)
    for core in range(N_CORES):
        b, j = divmod(core, CLS)
        out[b, j::CLS, :] = results[core]["out"]
    return out


def kernel(**inputs) -> np.ndarray:
    from concourse.bass_utils import run_bass_kernel_spmd

    in_maps = make_in_maps(**inputs)
    bva = np.frombuffer(
        in_maps[0]["pk"][0, OFF_BVA : OFF_BVA + VA * 2].tobytes(),
        dtype=mybir.dt.np(BF16),
    ).astype(np.float32)
    mask = np.ones(VA, bool)
    mask[HD::HDA] = False  # the ones columns
    nc = _get_program(bv_nonzero=bool(np.any(bva[mask] != 0.0)))
    res = run_bass_kernel_spmd(nc, in_maps, core_ids=list(range(N_CORES)))
    return assemble_output(res.results)


# revision 19
# speedup vs baseline: 2.0150x; 2.0150x over previous
"""Trainium2 Bass kernel for a dense transformer block (LN->causal attn->res->LN->MLP->res).

Shapes (hardcoded): x [2, 2048, 1024], 16 heads, head_dim 64, MLP hidden 4096, fp32 out.

v6 sharding: 8 cores = (batch b in {0,1}) x (token class j in {0..3}).
Class j = tokens {t : t % 4 == j} (512 tokens, position order).  Each core
gets the full 2048-token context CLASS-MAJOR with its own class LAST
(block order (j+1)%4, (j+2)%4, (j+3)%4, j), computes LN1 + K/V over the whole
context and Q for its own class, then runs causal-skip attention: any class
block's 128-token tile t covers positions [512t, 512(t+1)), so key tile
(block, t) is needed only by query tiles q >= t -- tiles with t > q are never
computed (37.5% of score/exp/AV work skipped, uniformly on every core; the
interleave balances the causal triangle).  The ragged diagonal (q == t) is a
per-core 0/1 mask (tri if the block's class <= own class else strictly-lower)
multiplied after the exp.  LN2/MLP/residuals for the own 512 tokens; the host
scatters rows back to positions j::4.  (A K/V AllGather variant was tried:
the 4-core gather costs ~120us/collective on this runtime and loses to the
~80us of duplicated K/V compute it saves.)

P1/P2 are fused into one batch pipeline: per 4-tile x batch, LN -> (Q for the
own batch) -> the K context chunk and V tiles that batch enables, so the PE
never waits for the whole LN pass (engines execute in program order; the
earlier phase-ordered version left the PE stalled behind not-yet-ready
transposes).  DMA issue order is tuned: batch-0 x goes out first (WVA after
it, trimask just before attention) so LN starts ~10us earlier, and the output
writes back in two 1MB halves as their columns finalize.

Carried over from v3/v4: ONE packed uint8 input tensor (runtime staging costs
~63us per tensor + ~10us/MB per exec, dominating the wall clock); bf16
weights (fp8 fails the 2e-2 error gate); bf16-only x; V augmented with a
per-head ones column accumulating the softmax denominator.
"""

from contextlib import ExitStack

import numpy as np

import concourse.bacc as bacc
import concourse.mybir as mybir
import concourse.tile as tile
from concourse.masks import make_identity

F32 = mybir.dt.float32
BF16 = mybir.dt.bfloat16
FP8 = mybir.dt.float8e4
AF = mybir.ActivationFunctionType
ALU = mybir.AluOpType

B = 2
T = 2048
D = 1024
H = 16
HD = 64
HDA = HD + 1  # +1 denominator column per head
MLP = 4096
NQ = 512  # tokens per core
CTX = T
EPS = 1e-5

N_CORES = 8
P = 128
CLS = 4

D_T = D // P  # 8
Q_T = NQ // P  # 4 query tiles (also key tiles per class)
M_T = MLP // P  # 32
VA = H * HDA  # 1040 augmented V width

REPLICA_GROUPS = [[0, 1, 2, 3], [4, 5, 6, 7]]

# ---- packed-input layout (bytes). All segments 4KB-aligned. ----
def _align(x, a=4096):
    return (x + a - 1) // a * a


_off = 0
def _seg(nbytes):
    global _off
    o = _off
    _off = _align(_off + nbytes)
    return o


OFF_X = _seg(CTX * D * 2)           # bf16 [2048,1024] class-major, own class last
OFF_WQ = _seg(D * D)                # fp8 [1024,1024] pretiled (x256)
OFF_WK = _seg(D * D)                # fp8 [1024,1024] pretiled (x256)
OFF_WVA = _seg(D * VA * 2)          # bf16 [1024,1040]
OFF_WFC = _seg(MLP * D * 2)         # bf16 [4096,1024] pretiled
OFF_WPJ = _seg(D * MLP * 2)         # bf16 [1024,4096] pretiled
OFF_TRI = _seg(P * CLS * 2 * P)     # fp8 0/1 [128, 4, 2, 128]
OFF_BQK = _seg(P * 2 * D_T * 4)     # f32 [128,16]
OFF_BFC = _seg(P * M_T * 4)         # f32 [128,32]
OFF_BPJ = _seg(P * D_T * 4)         # f32 [128,8]
OFF_BVA = _seg(VA * 2)              # bf16 [1,1040]
NB = _align(_off)


def build_program(loop_n: int = 1, bv_nonzero: bool = False):
    """Emit the SPMD Bass program. Returns finalized nc."""
    nc = bacc.Bacc("TRN2", target_bir_lowering=False)

    pk = nc.dram_tensor("pk", [1, NB], mybir.dt.uint8, kind="ExternalInput")
    out = nc.dram_tensor("out", [NQ, D], F32, kind="ExternalOutput")

    def view(off, nbytes, dt):
        return pk[0, off : off + nbytes].bitcast(dt)

    with tile.TileContext(nc) as tc:
        with ExitStack() as ctx:
            if loop_n > 1:
                ctx.enter_context(tc.For_i(0, loop_n, 1))
            const = ctx.enter_context(tc.tile_pool(name="const", bufs=1))
            identity = const.tile([P, P], F32)
            make_identity(nc, identity)
            identity_bf = const.tile([P, P], BF16)
            make_identity(nc, identity_bf)
            ones1 = const.tile([1, P], BF16)
            nc.vector.memset(ones1, 1.0)
            eps_t = const.tile([P, 1], F32)
            nc.vector.memset(eps_t, EPS)
            bqk_sb = const.tile([P, 2 * D_T], F32)
            nc.sync.dma_start(
                bqk_sb, view(OFF_BQK, P * 2 * D_T * 4, F32).rearrange("(p c) -> p c", p=P)
            )
            bva_sb = const.tile([1, VA], BF16)
            nc.sync.dma_start(
                bva_sb, view(OFF_BVA, VA * 2, BF16).rearrange("(p c) -> p c", p=1)
            )
            tri8 = const.tile([P, CLS, 2, P], FP8)
            tri_sb = const.tile([P, CLS, 2, P], BF16)

            # Long-lived pools.
            qt_cm = tc.tile_pool(name="qt", bufs=1)
            qt_pool = qt_cm.__enter__()
            QT = [qt_pool.tile([P, NQ], BF16, name=f"QT{i}") for i in range(D_T)]
            kt_cm = tc.tile_pool(name="ktp", bufs=1)
            kt_pool = kt_cm.__enter__()
            KTT = kt_pool.tile([P, D_T, CTX], BF16, name="KTT")
            vsb_cm = tc.tile_pool(name="vsb", bufs=1)
            vsb_pool = vsb_cm.__enter__()
            VSBT = vsb_pool.tile([P, CLS * Q_T, VA], BF16, name="VSBT")

            # RIGHT pools (live into P4/P5)
            yt_pool = ctx.enter_context(tc.tile_pool(name="yt", bufs=1, side="right"))
            YT = [yt_pool.tile([P, NQ], F32, name=f"YT{i}") for i in range(D_T)]
            x2_pool = ctx.enter_context(tc.tile_pool(name="x2", bufs=1, side="right"))
            X2 = [x2_pool.tile([P, D], F32, name=f"X2{i}") for i in range(Q_T)]
            l2t_pool = ctx.enter_context(
                tc.tile_pool(name="l2t", bufs=1, side="right")
            )
            L2T = [l2t_pool.tile([P, NQ], BF16, name=f"L2T{i}") for i in range(D_T)]
            xo_pool = ctx.enter_context(tc.tile_pool(name="xo", bufs=1, side="right"))
            XO = xo_pool.tile([P, Q_T, D], BF16, name="XO")
            # xnT on top of the RIGHT stack; freed after Q/K/V, wfc prefetch
            # reuses the space during attention.
            xnt_cm = tc.tile_pool(name="xnt", bufs=1, side="right")
            xnt_pool = xnt_cm.__enter__()
            xnT = [xnt_pool.tile([P, CTX], BF16, name=f"xnT{i}") for i in range(D_T)]

            # -------- P1+P2 fused: per x-batch LN -> (Q) -> K chunk -> V tiles
            # PE stays fed: batch 0 (own class, ctx cols 1536:2048) lands
            # first, then Q, then each later batch's LN overlaps the previous
            # batch's K/V matmuls.  wk streams per (nt, mt); WVA is resident.
            xcb_v = view(OFF_X, CTX * D * 2, BF16)
            batches = ((12, 13, 14, 15), (0, 1, 2, 3), (4, 5, 6, 7), (8, 9, 10, 11))
            ntof = (3, 0, 1, 2)  # K context chunk produced after each batch
            with tc.tile_pool(name="p2vw", bufs=1) as p2vw, tc.tile_pool(
                name="p1work", bufs=2
            ) as p1w, tc.tile_pool(name="p1xn", bufs=4) as p1xn, tc.tile_pool(
                name="p1stat", bufs=6
            ) as p1s, tc.tile_pool(name="p2w", bufs=2) as p2w, tc.tile_pool(
                name="p1ps", bufs=4, space="PSUM"
            ) as p1ps, tc.tile_pool(
                name="p2ps", bufs=2, space="PSUM"
            ) as p2ps, tc.tile_pool(
                name="p2vps", bufs=2, space="PSUM", side="right"
            ) as p2vps:
                WVA = p2vw.tile([P, D_T, VA], BF16, name="wva")
                vchunks = [(0, 512), (512, 512), (1024, VA - 1024)]
                for bi, bt in enumerate(batches):
                    if bi == 0:
                        xt = XO
                    else:
                        xt = p1w.tile([P, 4, D], BF16, tag="xt")
                    nc.sync.dma_start(
                        xt,
                        xcb_v[bt[0] * P * D : (bt[0] + 4) * P * D].rearrange(
                            "(a p c) -> p a c", p=P, c=D
                        ),
                    )
                    if bi == 0:
                        nc.sync.dma_start(
                            WVA,
                            view(OFF_WVA, D * VA * 2, BF16).rearrange(
                                "(a p c) -> p a c", p=P, c=VA
                            ),
                        )
                    xns = []
                    for ai, tt in enumerate(bt):
                        stats = p1s.tile([P, 2, 6], F32, tag="stats")
                        for g in range(2):
                            nc.vector.bn_stats(
                                stats[:, g, :], xt[:, ai, g * 512 : (g + 1) * 512]
                            )
                        mv = p1s.tile([P, 2], F32, tag="mv")
                        nc.vector.bn_aggr(mv, stats)
                        sd = p1s.tile([P, 1], F32, tag="sd")
                        nc.scalar.activation(sd, mv[:, 1:2], AF.Sqrt, bias=eps_t)
                        rstd = p1s.tile([P, 1], F32, tag="rstd")
                        nc.vector.reciprocal(rstd, sd)
                        nmb = p1s.tile([P, 1], F32, tag="nmb")
                        nc.vector.tensor_scalar(
                            nmb, mv[:, 0:1], rstd, -1.0, ALU.mult, ALU.mult
                        )
                        xn = p1xn.tile([P, D], BF16, tag="xn")
                        nc.scalar.activation(
                            xn, xt[:, ai, :], AF.Identity, bias=nmb, scale=rstd
                        )
                        xns.append(xn)
                    for dt_ in range(D_T):
                        tp = p1ps.tile([P, 4, P], BF16, tag="tp")
                        for ai in range(4):
                            nc.tensor.transpose(
                                tp[:, ai, :],
                                xns[ai][:, dt_ * P : (dt_ + 1) * P],
                                identity_bf,
                            )
                        if dt_ % 2 == 0:
                            nc.vector.tensor_copy(
                                xnT[dt_][:, bt[0] * P : (bt[0] + 4) * P], tp
                            )
                        else:
                            nc.scalar.copy(
                                xnT[dt_][:, bt[0] * P : (bt[0] + 4) * P], tp
                            )
                    nt = ntof[bi]
                    if bi == 0:
                        # Q^T for the own class (ctx cols 1536:2048)
                        for mt in range(D_T):
                            ws = p2w.tile([P, D_T, P], FP8, tag="wsk")
                            nc.sync.dma_start(
                                ws,
                                view(
                                    OFF_WQ + mt * P * D, P * D, FP8
                                ).rearrange("(p a c) -> p a c", p=P, c=P),
                            )
                            ps = p2ps.tile([P, NQ], F32, tag="ps")
                            for kt_ in range(D_T):
                                nc.tensor.matmul(
                                    ps,
                                    ws[:, kt_, :],
                                    xnT[kt_][:, CTX - NQ :],
                                    start=(kt_ == 0),
                                    stop=(kt_ == D_T - 1),
                                )
                            nc.scalar.activation(
                                QT[mt],
                                ps,
                                AF.Identity,
                                bias=bqk_sb[:, mt : mt + 1],
                                scale=1.0 / 256.0,
                            )
                    # K^T chunk nt (ctx cols nt*512 .. +512)
                    for mt in range(D_T):
                        ws = p2w.tile([P, D_T, P], FP8, tag="wsk")
                        nc.sync.dma_start(
                            ws,
                            view(OFF_WK + mt * P * D, P * D, FP8).rearrange(
                                "(p a c) -> p a c", p=P, c=P
                            ),
                        )
                        ps = p2ps.tile([P, 512], F32, tag="ps")
                        for kt_ in range(D_T):
                            nc.tensor.matmul(
                                ps,
                                ws[:, kt_, :],
                                xnT[kt_][:, nt * 512 : (nt + 1) * 512],
                                start=(kt_ == 0),
                                stop=(kt_ == D_T - 1),
                            )
                        nc.vector.tensor_scalar(
                            KTT[:, mt, nt * 512 : (nt + 1) * 512],
                            ps,
                            1.0 / 256.0,
                            bqk_sb[:, D_T + mt : D_T + mt + 1],
                            ALU.mult,
                            ALU.add,
                        )
                    # V_aug for this batch's 4 context tiles
                    for mtv in bt:
                        for ci, (c0, cw) in enumerate(vchunks):
                            ps = p2vps.tile([P, 512], F32, tag="ps")
                            for kt_ in range(D_T):
                                nc.tensor.matmul(
                                    ps[:, :cw],
                                    xnT[kt_][:, mtv * P : (mtv + 1) * P],
                                    WVA[:, kt_, c0 : c0 + cw],
                                    start=(kt_ == 0),
                                    stop=(kt_ == D_T - 1 and not bv_nonzero),
                                )
                            if bv_nonzero:
                                nc.tensor.matmul(
                                    ps[:, :cw],
                                    ones1,
                                    bva_sb[:, c0 : c0 + cw],
                                    start=False,
                                    stop=True,
                                )
                            if ci % 2 == 0:
                                nc.vector.tensor_copy(
                                    VSBT[:, mtv, c0 : c0 + cw], ps[:, :cw]
                                )
                            else:
                                nc.scalar.copy(
                                    VSBT[:, mtv, c0 : c0 + cw], ps[:, :cw]
                                )
                if not bv_nonzero:
                    ones_cols = VSBT.rearrange("p t (h c) -> p t h c", c=HDA)[
                        :, :, :, HD : HD + 1
                    ]
                    nc.vector.memset(ones_cols, 1.0)

            # xnT consumed -> free; prefetch half of wfc during attention.
            xnt_cm.__exit__(None, None, None)
            MT_RES = M_T // 2
            wfc_cm = tc.tile_pool(name="wfcp_sb", bufs=1, side="right")
            wfc_pool = wfc_cm.__enter__()
            WFC = wfc_pool.tile([P, MT_RES, D_T, P], BF16, name="WFC")
            nc.sync.dma_start(
                WFC,
                view(OFF_WFC, MT_RES * P * D * 2, BF16).rearrange(
                    "(a p c) -> p a c", p=P, c=D
                ).rearrange("p a (k c) -> p a k c", c=P),
            )

            # -------- P3: causal-skip attention, key tile (c, t) -------------
            # key tile (class c, tile t) serves query tiles q in [t, 4); the
            # first 128 query columns (q == t) get the ragged tri mask.
            nc.sync.dma_start(
                tri8,
                view(OFF_TRI, P * CLS * 2 * P, FP8).rearrange(
                    "(p c s q) -> p c s q", p=P, c=CLS, s=2
                ),
            )
            nc.vector.tensor_copy(tri_sb, tri8)
            ptp_cm = tc.tile_pool(name="ptp", bufs=4)
            ptp = ptp_cm.__enter__()
            p3s_cm = tc.tile_pool(name="p3s", bufs=2)
            p3s = p3s_cm.__enter__()
            stps_cm = tc.tile_pool(name="stps", bufs=2, space="PSUM")
            stps = stps_cm.__enter__()
            yps_cm = tc.tile_pool(name="yps", bufs=2, space="PSUM")
            yps = yps_cm.__enter__()
            for hp in range(H // 2):
                yp = yps.tile([HDA, 2, NQ], F32, name=f"yp{hp}", tag="yp")
                for t in range(Q_T):
                    nqc = (Q_T - t) * P  # query columns t*128 .. 512
                    for c in range(CLS):
                        kti = c * Q_T + t
                        kcol = c * NQ + t * P
                        # fixed 512-wide halves keep each matmul's PSUM
                        # region inside one 2KB bank
                        st = stps.tile([P, 2, NQ], F32, tag="st")
                        for s in range(2):
                            nc.tensor.matmul(
                                st[:, s, :nqc],
                                KTT[s * HD : (s + 1) * HD, hp, kcol : kcol + P],
                                QT[hp][s * HD : (s + 1) * HD, t * P :],
                                start=True,
                                stop=True,
                                tile_position=(s * HD, 0),
                            )
                        pt = ptp.tile([P, 2, nqc], BF16, tag="pt")
                        nc.scalar.activation(pt, st[:, :, :nqc], AF.Exp)
                        nc.vector.tensor_mul(
                            pt[:, :, 0:P], pt[:, :, 0:P], tri_sb[:, c, :, :]
                        )
                        # start=True zeroes the whole 2KB PSUM bank (one
                        # bank per s), so later sub-range accumulations are
                        # against zeroed/accumulated state; one start at
                        # (t=0,c=0), one stop at (t=3,c=3) per bank.
                        for s in range(2):
                            h = 2 * hp + s
                            nc.tensor.matmul(
                                yp[:, s, t * P :],
                                VSBT[:, kti, h * HDA : (h + 1) * HDA],
                                pt[:, s, :],
                                start=(t == 0 and c == 0),
                                stop=(t == Q_T - 1 and c == CLS - 1),
                            )
                for s in range(2):
                    ysb = p3s.tile([HDA, NQ], F32, name=f"ysb{hp}_{s}", tag="ysb")
                    if s == 0:
                        nc.vector.tensor_copy(ysb, yp[:, s, :])
                    else:
                        nc.scalar.copy(ysb, yp[:, s, :])
                    recip = p3s.tile([1, NQ], F32, tag="recip")
                    nc.vector.reciprocal(recip, ysb[HD : HD + 1, :])
                    rb = p3s.tile([HD, NQ], F32, tag="rb")
                    nc.gpsimd.partition_broadcast(rb, recip)
                    nc.vector.tensor_mul(
                        YT[hp][s * HD : (s + 1) * HD, :], ysb[:HD, :], rb
                    )

            yps_cm.__exit__(None, None, None)
            stps_cm.__exit__(None, None, None)
            p3s_cm.__exit__(None, None, None)
            ptp_cm.__exit__(None, None, None)
            vsb_cm.__exit__(None, None, None)
            kt_cm.__exit__(None, None, None)
            qt_cm.__exit__(None, None, None)

            # ---------------- P4: residual + LN2 + transpose -----------------
            with tc.tile_pool(name="p4w", bufs=3) as p4w, tc.tile_pool(
                name="p4s", bufs=4
            ) as p4s, tc.tile_pool(name="p4ps", bufs=4, space="PSUM") as p4ps:
                for tt in range(Q_T):
                    for mt in range(D_T):
                        tp = p4ps.tile([P, P], F32, tag="tp")
                        nc.tensor.transpose(
                            tp, YT[mt][:, tt * P : (tt + 1) * P], identity
                        )
                        nc.vector.tensor_add(
                            X2[tt][:, mt * P : (mt + 1) * P],
                            XO[:, tt, mt * P : (mt + 1) * P],
                            tp,
                        )
                    stats = p4s.tile([P, 2, 6], F32, tag="stats2")
                    for g in range(2):
                        nc.vector.bn_stats(
                            stats[:, g, :], X2[tt][:, g * 512 : (g + 1) * 512]
                        )
                    mv = p4s.tile([P, 2], F32, tag="mv2")
                    nc.vector.bn_aggr(mv, stats)
                    sd = p4s.tile([P, 1], F32, tag="sd2")
                    nc.scalar.activation(sd, mv[:, 1:2], AF.Sqrt, bias=eps_t)
                    rstd = p4s.tile([P, 1], F32, tag="rstd2")
                    nc.vector.reciprocal(rstd, sd)
                    nmb = p4s.tile([P, 1], F32, tag="nmb2")
                    nc.vector.tensor_scalar(
                        nmb, mv[:, 0:1], rstd, -1.0, ALU.mult, ALU.mult
                    )
                    l2 = p4w.tile([P, D], BF16, tag="l2")
                    nc.scalar.activation(l2, X2[tt], AF.Identity, bias=nmb, scale=rstd)
                    for mt in range(D_T):
                        tp = p4ps.tile([P, P], BF16, tag="tpb")
                        nc.tensor.transpose(
                            tp, l2[:, mt * P : (mt + 1) * P], identity_bf
                        )
                        if mt % 2 == 0:
                            nc.vector.tensor_copy(
                                L2T[mt][:, tt * P : (tt + 1) * P], tp
                            )
                        else:
                            nc.scalar.copy(L2T[mt][:, tt * P : (tt + 1) * P], tp)

            # ---------------- P5: MLP + final residual ----------------
            with tc.tile_pool(name="h1t", bufs=1) as h1t_pool, tc.tile_pool(
                name="p5w", bufs=2
            ) as p5w, tc.tile_pool(name="p5o", bufs=1) as p5o, tc.tile_pool(
                name="p5ps", bufs=3, space="PSUM"
            ) as p5ps, tc.tile_pool(
                name="p5tps", bufs=4, space="PSUM"
            ) as p5tps:
                bfc_sb = p5o.tile([P, M_T], F32)
                nc.sync.dma_start(
                    bfc_sb,
                    view(OFF_BFC, P * M_T * 4, F32).rearrange("(p c) -> p c", p=P),
                )
                bproj_sb = p5o.tile([P, D_T], F32)
                nc.sync.dma_start(
                    bproj_sb,
                    view(OFF_BPJ, P * D_T * 4, F32).rearrange("(p c) -> p c", p=P),
                )
                OUT = p5o.tile([P, Q_T, D], F32, name="OUT")
                H1T = [h1t_pool.tile([P, NQ], BF16, name=f"H1T{i}") for i in range(M_T)]
                for mt in range(M_T):
                    if mt < MT_RES:
                        wfc_t = WFC[:, mt]
                    else:
                        wfc_t = p5w.tile([P, D_T, P], BF16, tag="wsf")
                        nc.sync.dma_start(
                            wfc_t,
                            view(OFF_WFC + mt * P * D * 2, P * D * 2, BF16).rearrange(
                                "(p k c) -> p k c", p=P, c=P
                            ),
                        )
                    ps = p5ps.tile([P, NQ], F32, tag="ps")
                    for kt_ in range(D_T):
                        nc.tensor.matmul(
                            ps,
                            wfc_t[:, kt_, :],
                            L2T[kt_],
                            start=(kt_ == 0),
                            stop=(kt_ == D_T - 1),
                        )
                    nc.scalar.activation(
                        H1T[mt], ps, AF.Relu, bias=bfc_sb[:, mt : mt + 1]
                    )
                wfc_cm.__exit__(None, None, None)
                for mt in range(D_T):
                    ws = p5w.tile([P, M_T, P], BF16, tag="wsp")
                    nc.sync.dma_start(
                        ws,
                        view(OFF_WPJ + mt * P * MLP * 2, P * MLP * 2, BF16).rearrange(
                            "(p a c) -> p a c", p=P, c=P
                        ),
                    )
                    ps = p5ps.tile([P, NQ], F32, tag="ps")
                    for kt_ in range(M_T):
                        nc.tensor.matmul(
                            ps,
                            ws[:, kt_, :],
                            H1T[kt_],
                            start=(kt_ == 0),
                            stop=(kt_ == M_T - 1),
                        )
                    mlpt = p5w.tile([P, NQ], F32, tag="mlpt")
                    nc.vector.tensor_scalar_add(mlpt, ps, bproj_sb[:, mt : mt + 1])
                    for tt in range(Q_T):
                        tp = p5tps.tile([P, P], F32, tag="tp")
                        nc.tensor.transpose(
                            tp, mlpt[:, tt * P : (tt + 1) * P], identity
                        )
                        nc.vector.tensor_add(
                            OUT[:, tt, mt * P : (mt + 1) * P],
                            X2[tt][:, mt * P : (mt + 1) * P],
                            tp,
                        )
                    if mt == D_T // 2 - 1 or mt == D_T - 1:
                        h0 = 0 if mt < D_T // 2 else D // 2
                        nc.sync.dma_start(
                            out.rearrange("(a p) c -> p a c", p=P)[
                                :, :, h0 : h0 + D // 2
                            ],
                            OUT[:, :, h0 : h0 + D // 2],
                        )


    nc.finalize()
    return nc


_PROG = {}


def _get_program(bv_nonzero: bool = False):
    if bv_nonzero not in _PROG:
        _PROG[bv_nonzero] = build_program(bv_nonzero=bv_nonzero)
    return _PROG[bv_nonzero]


def _pretile(w, n_out_tiles, n_k_tiles):
    """[K, N] -> lhsT pre-tiled layout: row (mt*128+p), flat col (kt*128+c)
    holds w[kt*128 + p, mt*128 + c]."""
    K, N = w.shape
    assert K == n_k_tiles * P and N == n_out_tiles * P
    # axes (kt, p, mt, c) -> (mt, p, kt, c)
    return np.ascontiguousarray(
        w.reshape(n_k_tiles, P, n_out_tiles, P)
        .transpose(2, 1, 0, 3)
        .reshape(n_out_tiles * P, n_k_tiles * P)
    )


def make_in_maps(x, ln1_scale, ln1_shift, w_qkv, b_qkv, ln2_scale, ln2_shift,
                 w_fc, b_fc, w_proj, b_proj):
    """Host-side prep: fold LN affine into weights, prescale Q by 1/sqrt(hd),
    augment V with the ones column, pre-tile weights (bf16), build the
    per-core class-interleaved x slice + ragged-diagonal masks, and pack
    everything into one uint8 tensor per core."""
    import ml_dtypes

    bf16 = ml_dtypes.bfloat16
    fp8 = mybir.dt.np(FP8)

    x = np.asarray(x, np.float32)
    ln1_scale = np.asarray(ln1_scale, np.float32)
    ln1_shift = np.asarray(ln1_shift, np.float32)
    w_qkv = np.asarray(w_qkv, np.float32)
    b_qkv = np.asarray(b_qkv, np.float32)
    ln2_scale = np.asarray(ln2_scale, np.float32)
    ln2_shift = np.asarray(ln2_shift, np.float32)
    w_fc = np.asarray(w_fc, np.float32)
    b_fc = np.asarray(b_fc, np.float32)
    w_proj = np.asarray(w_proj, np.float32)
    b_proj = np.asarray(b_proj, np.float32)

    # fold LN1 affine into qkv weights
    w1 = ln1_scale[:, None] * w_qkv  # [D, 3D]
    b1 = b_qkv + ln1_shift @ w_qkv  # [3D]
    sc = 1.0 / np.sqrt(HD)
    wq = w1[:, :D] * sc
    bq = b1[:D] * sc
    wk = w1[:, D : 2 * D]
    bk = b1[D : 2 * D]
    wv = w1[:, 2 * D :]
    bv = b1[2 * D :]

    wqp_h = (_pretile(wq, D_T, D_T) * 256.0).astype(fp8)
    wkp_h = (_pretile(wk, D_T, D_T) * 256.0).astype(fp8)
    bqk_h = np.ascontiguousarray(
        np.concatenate([bq, bk]).reshape(2 * D_T, P).T
    )  # [128, 16] f32

    wva_h = np.zeros((D, VA), np.float32)
    bva_h = np.zeros((1, VA), np.float32)
    for h in range(H):
        wva_h[:, h * HDA : h * HDA + HD] = wv[:, h * HD : (h + 1) * HD]
        bva_h[0, h * HDA : h * HDA + HD] = bv[h * HD : (h + 1) * HD]
        bva_h[0, h * HDA + HD] = 1.0  # denominator ones column
    wva_h = wva_h.astype(bf16)
    bva_h = bva_h.astype(bf16)

    # fold LN2 affine into fc; pre-tile bf16
    wfc_f = ln2_scale[:, None] * w_fc
    wfcp_h = _pretile(wfc_f, M_T, D_T).astype(bf16)  # [4096, 1024]
    wprojp_h = _pretile(w_proj, D_T, M_T).astype(bf16)  # [1024, 4096]
    bfc_h = np.ascontiguousarray((b_fc + ln2_shift @ w_fc).reshape(M_T, P).T)
    bproj_h = np.ascontiguousarray(b_proj.reshape(D_T, P).T)  # [128, 8]

    def put(buf, off, arr):
        bts = np.ascontiguousarray(arr).view(np.uint8).reshape(-1)
        buf[off : off + bts.size] = bts

    base = np.zeros(NB, np.uint8)
    put(base, OFF_WQ, wqp_h)
    put(base, OFF_WK, wkp_h)
    put(base, OFF_WVA, wva_h)
    put(base, OFF_WFC, wfcp_h)
    put(base, OFF_WPJ, wprojp_h)
    put(base, OFF_BQK, bqk_h)
    put(base, OFF_BFC, bfc_h)
    put(base, OFF_BPJ, bproj_h)
    put(base, OFF_BVA, bva_h)

    # ragged diagonal masks: key index i (partition) vs query index col:
    # keep if i <= col (key class c <= own class j) else i < col.
    ii = np.arange(P)[:, None]
    qq = np.arange(P)[None, :]
    tri_inc = (ii <= qq).astype(np.float32)  # [128,128]
    tri_exc = (ii < qq).astype(np.float32)

    in_maps = []
    for core in range(N_CORES):
        b, j = divmod(core, CLS)
        # class-major context, own class last: block bi holds class (j+1+bi)%4
        blocks = [(j + 1 + bi) % CLS for bi in range(CLS)]
        xperm = np.concatenate([x[b, c::CLS] for c in blocks], axis=0)  # [2048, D]
        tri_h = np.empty((P, CLS, 2, P), np.float32)
        for bi, c in enumerate(blocks):
            m = tri_inc if c <= j else tri_exc
            tri_h[:, bi, 0, :] = m
            tri_h[:, bi, 1, :] = m
        pkc = base.copy()
        put(pkc, OFF_X, np.ascontiguousarray(xperm.astype(bf16)))
        put(pkc, OFF_TRI, tri_h.astype(fp8))
        in_maps.append({"pk": pkc.reshape(1, NB)})
    return in_maps


def assemble_output(results):
    out = np.empty((B, T, D), np.float32)
    for core in range(N_CORES):
        b, j = divmod(core, CLS)
        out[b, j::CLS, :] = results[core]["out"]
    return out


def kernel(**inputs) -> np.ndarray:
    from concourse.bass_utils import run_bass_kernel_spmd

    in_maps = make_in_maps(**inputs)
    bva = np.frombuffer(
        in_maps[0]["pk"][0, OFF_BVA : OFF_BVA + VA * 2].tobytes(),
        dtype=mybir.dt.np(BF16),
    ).astype(np.float32)
    mask = np.ones(VA, bool)
    mask[HD::HDA] = False  # the ones columns
    nc = _get_program(bv_nonzero=bool(np.any(bva[mask] != 0.0)))
    res = run_bass_kernel_spmd(nc, in_maps, core_ids=list(range(N_CORES)))
    return assemble_output(res.results)


# revision 28
# speedup vs baseline: 2.8052x; 1.3922x over previous
"""Trainium2 Bass kernel for a dense transformer block (LN->causal attn->res->LN->MLP->res).

Shapes (hardcoded): x [2, 2048, 1024], 16 heads, head_dim 64, MLP hidden 4096, fp32 out.

v6 sharding: 8 cores = (batch b in {0,1}) x (token class j in {0..3}).
Class j = tokens {t : t % 4 == j} (512 tokens, position order).  Each core
gets the full 2048-token context CLASS-MAJOR with its own class LAST
(block order (j+1)%4, (j+2)%4, (j+3)%4, j), computes LN1 + K/V over the whole
context and Q for its own class, then runs causal-skip attention: any class
block's 128-token tile t covers positions [512t, 512(t+1)), so key tile
(block, t) is needed only by query tiles q >= t -- tiles with t > q are never
computed (37.5% of score/exp/AV work skipped, uniformly on every core; the
interleave balances the causal triangle).  The ragged diagonal (q == t) is a
per-core 0/1 mask (tri if the block's class <= own class else strictly-lower)
multiplied after the exp.  LN2/MLP/residuals for the own 512 tokens; the host
scatters rows back to positions j::4.  (A K/V AllGather variant was tried:
the 4-core gather costs ~120us/collective on this runtime and loses to the
~80us of duplicated K/V compute it saves.)

P1/P2 are fused into one batch pipeline: per 4-tile x batch, LN -> (Q for the
own batch) -> the K context chunk and V tiles that batch enables, so the PE
never waits for the whole LN pass (engines execute in program order; the
earlier phase-ordered version left the PE stalled behind not-yet-ready
transposes).  DMA issue order is tuned: batch-0 x goes out first (WVA after
it, trimask just before attention) so LN starts ~10us earlier, and the output
writes back in two 1MB halves as their columns finalize.

Carried over from v3/v4: ONE packed uint8 input tensor (runtime staging costs
~63us per tensor + ~10us/MB per exec, dominating the wall clock); wq/wk
shipped fp8e4m3 x256 (their quantization only perturbs pre-softmax scores:
+~4e-3 rel err, fine) with 1/256 folded into the Q/K epilogues; wv/wfc/wproj
stay bf16 (fp8 there costs ~1e-2 each and fails the 2e-2 gate); bf16-only x;
V augmented with a per-head ones column accumulating the softmax denominator.
Output ships bf16 (half-ulp <=2e-3 on |out|max 6.6; host upcasts) to halve
output staging.
"""

from contextlib import ExitStack

import numpy as np

import concourse.bacc as bacc
import concourse.mybir as mybir
import concourse.tile as tile
from concourse.masks import make_identity

F32 = mybir.dt.float32
BF16 = mybir.dt.bfloat16
FP8 = mybir.dt.float8e4
AF = mybir.ActivationFunctionType
ALU = mybir.AluOpType

B = 2
T = 2048
D = 1024
H = 16
HD = 64
HDA = HD + 1  # +1 denominator column per head
MLP = 4096
NQ = 512  # tokens per core
CTX = T
EPS = 1e-5

N_CORES = 8
P = 128
CLS = 4

D_T = D // P  # 8
Q_T = NQ // P  # 4 query tiles (also key tiles per class)
M_T = MLP // P  # 32
VA = H * HDA  # 1040 augmented V width

REPLICA_GROUPS = [[0, 1, 2, 3], [4, 5, 6, 7]]

# ---- packed-input layout (bytes). All segments 4KB-aligned. ----
def _align(x, a=4096):
    return (x + a - 1) // a * a


_off = 0
def _seg(nbytes):
    global _off
    o = _off
    _off = _align(_off + nbytes)
    return o


OFF_X = _seg(CTX * D * 2)           # bf16 [2048,1024] class-major, own class last
OFF_WQ = _seg(D * D)                # fp8 [1024,1024] pretiled (x256)
OFF_WK = _seg(D * D)                # fp8 [1024,1024] pretiled (x256)
OFF_WVA = _seg(D * VA * 2)          # bf16 [1024,1040]
OFF_WFC = _seg(MLP * D * 2)         # bf16 [4096,1024] pretiled
OFF_WPJ = _seg(D * MLP * 2)         # bf16 [1024,4096] pretiled
OFF_TRI = _seg(P * CLS * 2 * P)     # fp8 0/1 [128, 4, 2, 128]
OFF_BQK = _seg(P * 2 * D_T * 4)     # f32 [128,16]
OFF_BFC = _seg(P * M_T * 4)         # f32 [128,32]
OFF_BPJ = _seg(P * D_T * 4)         # f32 [128,8]
OFF_BVA = _seg(VA * 2)              # bf16 [1,1040]
NB = _align(_off)


def build_program(loop_n: int = 1, bv_nonzero: bool = False):
    """Emit the SPMD Bass program. Returns finalized nc."""
    nc = bacc.Bacc("TRN2", target_bir_lowering=False)

    pk = nc.dram_tensor("pk", [1, NB], mybir.dt.uint8, kind="ExternalInput")
    out = nc.dram_tensor("out", [NQ, D], BF16, kind="ExternalOutput")

    def view(off, nbytes, dt):
        return pk[0, off : off + nbytes].bitcast(dt)

    with tile.TileContext(nc) as tc:
        with ExitStack() as ctx:
            if loop_n > 1:
                ctx.enter_context(tc.For_i(0, loop_n, 1))
            const = ctx.enter_context(tc.tile_pool(name="const", bufs=1))
            identity = const.tile([P, P], F32)
            make_identity(nc, identity)
            identity_bf = const.tile([P, P], BF16)
            make_identity(nc, identity_bf)
            ones1 = const.tile([1, P], BF16)
            nc.vector.memset(ones1, 1.0)
            eps_t = const.tile([P, 1], F32)
            nc.vector.memset(eps_t, EPS)
            bqk_sb = const.tile([P, 2 * D_T], F32)
            nc.sync.dma_start(
                bqk_sb, view(OFF_BQK, P * 2 * D_T * 4, F32).rearrange("(p c) -> p c", p=P)
            )
            bva_sb = const.tile([1, VA], BF16)
            nc.sync.dma_start(
                bva_sb, view(OFF_BVA, VA * 2, BF16).rearrange("(p c) -> p c", p=1)
            )
            tri8 = const.tile([P, CLS, 2, P], FP8)
            tri_sb = const.tile([P, CLS, 2, P], BF16)

            # Long-lived pools.
            qt_cm = tc.tile_pool(name="qt", bufs=1)
            qt_pool = qt_cm.__enter__()
            QT = [qt_pool.tile([P, NQ], BF16, name=f"QT{i}") for i in range(D_T)]
            kt_cm = tc.tile_pool(name="ktp", bufs=1)
            kt_pool = kt_cm.__enter__()
            KTT = kt_pool.tile([P, D_T, CTX], BF16, name="KTT")
            vsb_cm = tc.tile_pool(name="vsb", bufs=1)
            vsb_pool = vsb_cm.__enter__()
            VSBT = vsb_pool.tile([P, CLS * Q_T, VA], BF16, name="VSBT")

            # RIGHT pools (live into P4/P5)
            yt_pool = ctx.enter_context(tc.tile_pool(name="yt", bufs=1, side="right"))
            YT = [yt_pool.tile([P, NQ], F32, name=f"YT{i}") for i in range(D_T)]
            x2_pool = ctx.enter_context(tc.tile_pool(name="x2", bufs=1, side="right"))
            X2 = [x2_pool.tile([P, D], F32, name=f"X2{i}") for i in range(Q_T)]
            l2t_pool = ctx.enter_context(
                tc.tile_pool(name="l2t", bufs=1, side="right")
            )
            L2T = [l2t_pool.tile([P, NQ], BF16, name=f"L2T{i}") for i in range(D_T)]
            xo_pool = ctx.enter_context(tc.tile_pool(name="xo", bufs=1, side="right"))
            XO = xo_pool.tile([P, Q_T, D], BF16, name="XO")
            # xnT on top of the RIGHT stack; freed after Q/K/V, wfc prefetch
            # reuses the space during attention.
            xnt_cm = tc.tile_pool(name="xnt", bufs=1, side="right")
            xnt_pool = xnt_cm.__enter__()
            xnT = [xnt_pool.tile([P, CTX], BF16, name=f"xnT{i}") for i in range(D_T)]

            # -------- P1+P2 fused: per x-batch LN -> (Q) -> K chunk -> V tiles
            # PE stays fed: batch 0 (own class, ctx cols 1536:2048) lands
            # first, then Q, then each later batch's LN overlaps the previous
            # batch's K/V matmuls.  wk streams per (nt, mt); WVA is resident.
            xcb_v = view(OFF_X, CTX * D * 2, BF16)
            batches = ((12, 13, 14, 15), (0, 1, 2, 3), (4, 5, 6, 7), (8, 9, 10, 11))
            ntof = (3, 0, 1, 2)  # K context chunk produced after each batch
            with tc.tile_pool(name="p2vw", bufs=1) as p2vw, tc.tile_pool(
                name="p1work", bufs=2
            ) as p1w, tc.tile_pool(name="p1xn", bufs=4) as p1xn, tc.tile_pool(
                name="p1stat", bufs=6
            ) as p1s, tc.tile_pool(name="p2w", bufs=2) as p2w, tc.tile_pool(
                name="p1ps", bufs=4, space="PSUM"
            ) as p1ps, tc.tile_pool(
                name="p2ps", bufs=2, space="PSUM"
            ) as p2ps, tc.tile_pool(
                name="p2vps", bufs=2, space="PSUM", side="right"
            ) as p2vps:
                WVA = p2vw.tile([P, D_T, VA], BF16, name="wva")
                vchunks = [(0, 512), (512, 512), (1024, VA - 1024)]
                for bi, bt in enumerate(batches):
                    if bi == 0:
                        xt = XO
                    else:
                        xt = p1w.tile([P, 4, D], BF16, tag="xt")
                    nc.sync.dma_start(
                        xt,
                        xcb_v[bt[0] * P * D : (bt[0] + 4) * P * D].rearrange(
                            "(a p c) -> p a c", p=P, c=D
                        ),
                    )
                    if bi == 0:
                        nc.sync.dma_start(
                            WVA,
                            view(OFF_WVA, D * VA * 2, BF16).rearrange(
                                "(a p c) -> p a c", p=P, c=VA
                            ),
                        )
                    xns = []
                    for ai, tt in enumerate(bt):
                        stats = p1s.tile([P, 2, 6], F32, tag="stats")
                        for g in range(2):
                            nc.vector.bn_stats(
                                stats[:, g, :], xt[:, ai, g * 512 : (g + 1) * 512]
                            )
                        mv = p1s.tile([P, 2], F32, tag="mv")
                        nc.vector.bn_aggr(mv, stats)
                        sd = p1s.tile([P, 1], F32, tag="sd")
                        nc.scalar.activation(sd, mv[:, 1:2], AF.Sqrt, bias=eps_t)
                        rstd = p1s.tile([P, 1], F32, tag="rstd")
                        nc.vector.reciprocal(rstd, sd)
                        nmb = p1s.tile([P, 1], F32, tag="nmb")
                        nc.vector.tensor_scalar(
                            nmb, mv[:, 0:1], rstd, -1.0, ALU.mult, ALU.mult
                        )
                        xn = p1xn.tile([P, D], BF16, tag="xn")
                        nc.scalar.activation(
                            xn, xt[:, ai, :], AF.Identity, bias=nmb, scale=rstd
                        )
                        xns.append(xn)
                    for dt_ in range(D_T):
                        tp = p1ps.tile([P, 4, P], BF16, tag="tp")
                        for ai in range(4):
                            nc.tensor.transpose(
                                tp[:, ai, :],
                                xns[ai][:, dt_ * P : (dt_ + 1) * P],
                                identity_bf,
                            )
                        if dt_ % 2 == 0:
                            nc.vector.tensor_copy(
                                xnT[dt_][:, bt[0] * P : (bt[0] + 4) * P], tp
                            )
                        else:
                            nc.scalar.copy(
                                xnT[dt_][:, bt[0] * P : (bt[0] + 4) * P], tp
                            )
                    nt = ntof[bi]
                    if bi == 0:
                        # Q^T for the own class (ctx cols 1536:2048)
                        for mt in range(D_T):
                            ws = p2w.tile([P, D_T, P], FP8, tag="wsk")
                            nc.sync.dma_start(
                                ws,
                                view(
                                    OFF_WQ + mt * P * D, P * D, FP8
                                ).rearrange("(p a c) -> p a c", p=P, c=P),
                            )
                            ps = p2ps.tile([P, NQ], F32, tag="ps")
                            for kt_ in range(D_T):
                                nc.tensor.matmul(
                                    ps,
                                    ws[:, kt_, :],
                                    xnT[kt_][:, CTX - NQ :],
                                    start=(kt_ == 0),
                                    stop=(kt_ == D_T - 1),
                                )
                            nc.scalar.activation(
                                QT[mt],
                                ps,
                                AF.Identity,
                                bias=bqk_sb[:, mt : mt + 1],
                                scale=1.0 / 256.0,
                            )
                    # K^T chunk nt (ctx cols nt*512 .. +512)
                    for mt in range(D_T):
                        ws = p2w.tile([P, D_T, P], FP8, tag="wsk")
                        nc.sync.dma_start(
                            ws,
                            view(OFF_WK + mt * P * D, P * D, FP8).rearrange(
                                "(p a c) -> p a c", p=P, c=P
                            ),
                        )
                        ps = p2ps.tile([P, 512], F32, tag="ps")
                        for kt_ in range(D_T):
                            nc.tensor.matmul(
                                ps,
                                ws[:, kt_, :],
                                xnT[kt_][:, nt * 512 : (nt + 1) * 512],
                                start=(kt_ == 0),
                                stop=(kt_ == D_T - 1),
                            )
                        nc.vector.tensor_scalar(
                            KTT[:, mt, nt * 512 : (nt + 1) * 512],
                            ps,
                            1.0 / 256.0,
                            bqk_sb[:, D_T + mt : D_T + mt + 1],
                            ALU.mult,
                            ALU.add,
                        )
                    # V_aug for this batch's 4 context tiles
                    for mtv in bt:
                        for ci, (c0, cw) in enumerate(vchunks):
                            ps = p2vps.tile([P, 512], F32, tag="ps")
                            for kt_ in range(D_T):
                                nc.tensor.matmul(
                                    ps[:, :cw],
                                    xnT[kt_][:, mtv * P : (mtv + 1) * P],
                                    WVA[:, kt_, c0 : c0 + cw],
                                    start=(kt_ == 0),
                                    stop=(kt_ == D_T - 1 and not bv_nonzero),
                                )
                            if bv_nonzero:
                                nc.tensor.matmul(
                                    ps[:, :cw],
                                    ones1,
                                    bva_sb[:, c0 : c0 + cw],
                                    start=False,
                                    stop=True,
                                )
                            if ci % 2 == 0:
                                nc.vector.tensor_copy(
                                    VSBT[:, mtv, c0 : c0 + cw], ps[:, :cw]
                                )
                            else:
                                nc.scalar.copy(
                                    VSBT[:, mtv, c0 : c0 + cw], ps[:, :cw]
                                )
                if not bv_nonzero:
                    ones_cols = VSBT.rearrange("p t (h c) -> p t h c", c=HDA)[
                        :, :, :, HD : HD + 1
                    ]
                    nc.vector.memset(ones_cols, 1.0)

            # xnT consumed -> free; prefetch half of wfc during attention.
            xnt_cm.__exit__(None, None, None)
            MT_RES = M_T // 2
            wfc_cm = tc.tile_pool(name="wfcp_sb", bufs=1, side="right")
            wfc_pool = wfc_cm.__enter__()
            WFC = wfc_pool.tile([P, MT_RES, D_T, P], BF16, name="WFC")
            nc.sync.dma_start(
                WFC,
                view(OFF_WFC, MT_RES * P * D * 2, BF16).rearrange(
                    "(a p c) -> p a c", p=P, c=D
                ).rearrange("p a (k c) -> p a k c", c=P),
            )

            # -------- P3: causal-skip attention, key tile (c, t) -------------
            # key tile (class c, tile t) serves query tiles q in [t, 4); the
            # first 128 query columns (q == t) get the ragged tri mask.
            nc.sync.dma_start(
                tri8,
                view(OFF_TRI, P * CLS * 2 * P, FP8).rearrange(
                    "(p c s q) -> p c s q", p=P, c=CLS, s=2
                ),
            )
            nc.vector.tensor_copy(tri_sb, tri8)
            ptp_cm = tc.tile_pool(name="ptp", bufs=4)
            ptp = ptp_cm.__enter__()
            p3s_cm = tc.tile_pool(name="p3s", bufs=2)
            p3s = p3s_cm.__enter__()
            stps_cm = tc.tile_pool(name="stps", bufs=2, space="PSUM")
            stps = stps_cm.__enter__()
            yps_cm = tc.tile_pool(name="yps", bufs=2, space="PSUM")
            yps = yps_cm.__enter__()
            for hp in range(H // 2):
                yp = yps.tile([HDA, 2, NQ], F32, name=f"yp{hp}", tag="yp")
                for t in range(Q_T):
                    nqc = (Q_T - t) * P  # query columns t*128 .. 512
                    for c in range(CLS):
                        kti = c * Q_T + t
                        kcol = c * NQ + t * P
                        # fixed 512-wide halves keep each matmul's PSUM
                        # region inside one 2KB bank
                        st = stps.tile([P, 2, NQ], F32, tag="st")
                        for s in range(2):
                            nc.tensor.matmul(
                                st[:, s, :nqc],
                                KTT[s * HD : (s + 1) * HD, hp, kcol : kcol + P],
                                QT[hp][s * HD : (s + 1) * HD, t * P :],
                                start=True,
                                stop=True,
                                tile_position=(s * HD, 0),
                            )
                        pt = ptp.tile([P, 2, nqc], BF16, tag="pt")
                        nc.scalar.activation(pt, st[:, :, :nqc], AF.Exp)
                        nc.vector.tensor_mul(
                            pt[:, :, 0:P], pt[:, :, 0:P], tri_sb[:, c, :, :]
                        )
                        # start=True zeroes the whole 2KB PSUM bank (one
                        # bank per s), so later sub-range accumulations are
                        # against zeroed/accumulated state; one start at
                        # (t=0,c=0), one stop at (t=3,c=3) per bank.
                        for s in range(2):
                            h = 2 * hp + s
                            nc.tensor.matmul(
                                yp[:, s, t * P :],
                                VSBT[:, kti, h * HDA : (h + 1) * HDA],
                                pt[:, s, :],
                                start=(t == 0 and c == 0),
                                stop=(t == Q_T - 1 and c == CLS - 1),
                            )
                for s in range(2):
                    ysb = p3s.tile([HDA, NQ], F32, name=f"ysb{hp}_{s}", tag="ysb")
                    if s == 0:
                        nc.vector.tensor_copy(ysb, yp[:, s, :])
                    else:
                        nc.scalar.copy(ysb, yp[:, s, :])
                    recip = p3s.tile([1, NQ], F32, tag="recip")
                    nc.vector.reciprocal(recip, ysb[HD : HD + 1, :])
                    rb = p3s.tile([HD, NQ], F32, tag="rb")
                    nc.gpsimd.partition_broadcast(rb, recip)
                    nc.vector.tensor_mul(
                        YT[hp][s * HD : (s + 1) * HD, :], ysb[:HD, :], rb
                    )

            yps_cm.__exit__(None, None, None)
            stps_cm.__exit__(None, None, None)
            p3s_cm.__exit__(None, None, None)
            ptp_cm.__exit__(None, None, None)
            vsb_cm.__exit__(None, None, None)
            kt_cm.__exit__(None, None, None)
            qt_cm.__exit__(None, None, None)

            # ---------------- P4: residual + LN2 + transpose -----------------
            with tc.tile_pool(name="p4w", bufs=3) as p4w, tc.tile_pool(
                name="p4s", bufs=4
            ) as p4s, tc.tile_pool(name="p4ps", bufs=4, space="PSUM") as p4ps:
                for tt in range(Q_T):
                    for mt in range(D_T):
                        tp = p4ps.tile([P, P], F32, tag="tp")
                        nc.tensor.transpose(
                            tp, YT[mt][:, tt * P : (tt + 1) * P], identity
                        )
                        nc.vector.tensor_add(
                            X2[tt][:, mt * P : (mt + 1) * P],
                            XO[:, tt, mt * P : (mt + 1) * P],
                            tp,
                        )
                    stats = p4s.tile([P, 2, 6], F32, tag="stats2")
                    for g in range(2):
                        nc.vector.bn_stats(
                            stats[:, g, :], X2[tt][:, g * 512 : (g + 1) * 512]
                        )
                    mv = p4s.tile([P, 2], F32, tag="mv2")
                    nc.vector.bn_aggr(mv, stats)
                    sd = p4s.tile([P, 1], F32, tag="sd2")
                    nc.scalar.activation(sd, mv[:, 1:2], AF.Sqrt, bias=eps_t)
                    rstd = p4s.tile([P, 1], F32, tag="rstd2")
                    nc.vector.reciprocal(rstd, sd)
                    nmb = p4s.tile([P, 1], F32, tag="nmb2")
                    nc.vector.tensor_scalar(
                        nmb, mv[:, 0:1], rstd, -1.0, ALU.mult, ALU.mult
                    )
                    l2 = p4w.tile([P, D], BF16, tag="l2")
                    nc.scalar.activation(l2, X2[tt], AF.Identity, bias=nmb, scale=rstd)
                    for mt in range(D_T):
                        tp = p4ps.tile([P, P], BF16, tag="tpb")
                        nc.tensor.transpose(
                            tp, l2[:, mt * P : (mt + 1) * P], identity_bf
                        )
                        if mt % 2 == 0:
                            nc.vector.tensor_copy(
                                L2T[mt][:, tt * P : (tt + 1) * P], tp
                            )
                        else:
                            nc.scalar.copy(L2T[mt][:, tt * P : (tt + 1) * P], tp)

            # ---------------- P5: MLP + final residual ----------------
            with tc.tile_pool(name="h1t", bufs=1) as h1t_pool, tc.tile_pool(
                name="p5w", bufs=2
            ) as p5w, tc.tile_pool(name="p5o", bufs=1) as p5o, tc.tile_pool(
                name="p5ps", bufs=3, space="PSUM"
            ) as p5ps, tc.tile_pool(
                name="p5tps", bufs=4, space="PSUM"
            ) as p5tps:
                bfc_sb = p5o.tile([P, M_T], F32)
                nc.sync.dma_start(
                    bfc_sb,
                    view(OFF_BFC, P * M_T * 4, F32).rearrange("(p c) -> p c", p=P),
                )
                bproj_sb = p5o.tile([P, D_T], F32)
                nc.sync.dma_start(
                    bproj_sb,
                    view(OFF_BPJ, P * D_T * 4, F32).rearrange("(p c) -> p c", p=P),
                )
                OUT = p5o.tile([P, Q_T, D], BF16, name="OUT")
                H1T = [h1t_pool.tile([P, NQ], BF16, name=f"H1T{i}") for i in range(M_T)]
                for mt in range(M_T):
                    if mt < MT_RES:
                        wfc_t = WFC[:, mt]
                    else:
                        wfc_t = p5w.tile([P, D_T, P], BF16, tag="wsf")
                        nc.sync.dma_start(
                            wfc_t,
                            view(OFF_WFC + mt * P * D * 2, P * D * 2, BF16).rearrange(
                                "(p k c) -> p k c", p=P, c=P
                            ),
                        )
                    ps = p5ps.tile([P, NQ], F32, tag="ps")
                    for kt_ in range(D_T):
                        nc.tensor.matmul(
                            ps,
                            wfc_t[:, kt_, :],
                            L2T[kt_],
                            start=(kt_ == 0),
                            stop=(kt_ == D_T - 1),
                        )
                    nc.scalar.activation(
                        H1T[mt], ps, AF.Relu, bias=bfc_sb[:, mt : mt + 1]
                    )
                wfc_cm.__exit__(None, None, None)
                for mt in range(D_T):
                    ws = p5w.tile([P, M_T, P], BF16, tag="wsp")
                    nc.sync.dma_start(
                        ws,
                        view(OFF_WPJ + mt * P * MLP * 2, P * MLP * 2, BF16).rearrange(
                            "(p a c) -> p a c", p=P, c=P
                        ),
                    )
                    ps = p5ps.tile([P, NQ], F32, tag="ps")
                    for kt_ in range(M_T):
                        nc.tensor.matmul(
                            ps,
                            ws[:, kt_, :],
                            H1T[kt_],
                            start=(kt_ == 0),
                            stop=(kt_ == M_T - 1),
                        )
                    mlpt = p5w.tile([P, NQ], F32, tag="mlpt")
                    nc.vector.tensor_scalar_add(mlpt, ps, bproj_sb[:, mt : mt + 1])
                    for tt in range(Q_T):
                        tp = p5tps.tile([P, P], F32, tag="tp")
                        nc.tensor.transpose(
                            tp, mlpt[:, tt * P : (tt + 1) * P], identity
                        )
                        nc.vector.tensor_add(
                            OUT[:, tt, mt * P : (mt + 1) * P],
                            X2[tt][:, mt * P : (mt + 1) * P],
                            tp,
                        )
                    if mt == D_T // 2 - 1 or mt == D_T - 1:
                        h0 = 0 if mt < D_T // 2 else D // 2
                        nc.sync.dma_start(
                            out.rearrange("(a p) c -> p a c", p=P)[
                                :, :, h0 : h0 + D // 2
                            ],
                            OUT[:, :, h0 : h0 + D // 2],
                        )


    nc.finalize()
    return nc


_PROG = {}


def _get_program(bv_nonzero: bool = False):
    if bv_nonzero not in _PROG:
        _PROG[bv_nonzero] = build_program(bv_nonzero=bv_nonzero)
    return _PROG[bv_nonzero]


def _pretile(w, n_out_tiles, n_k_tiles):
    """[K, N] -> lhsT pre-tiled layout: row (mt*128+p), flat col (kt*128+c)
    holds w[kt*128 + p, mt*128 + c]."""
    K, N = w.shape
    assert K == n_k_tiles * P and N == n_out_tiles * P
    # axes (kt, p, mt, c) -> (mt, p, kt, c)
    return np.ascontiguousarray(
        w.reshape(n_k_tiles, P, n_out_tiles, P)
        .transpose(2, 1, 0, 3)
        .reshape(n_out_tiles * P, n_k_tiles * P)
    )


def make_in_maps(x, ln1_scale, ln1_shift, w_qkv, b_qkv, ln2_scale, ln2_shift,
                 w_fc, b_fc, w_proj, b_proj):
    """Host-side prep: fold LN affine into weights, prescale Q by 1/sqrt(hd),
    augment V with the ones column, pre-tile weights (bf16), build the
    per-core class-interleaved x slice + ragged-diagonal masks, and pack
    everything into one uint8 tensor per core."""
    import ml_dtypes

    bf16 = ml_dtypes.bfloat16
    fp8 = mybir.dt.np(FP8)

    x = np.asarray(x, np.float32)
    ln1_scale = np.asarray(ln1_scale, np.float32)
    ln1_shift = np.asarray(ln1_shift, np.float32)
    w_qkv = np.asarray(w_qkv, np.float32)
    b_qkv = np.asarray(b_qkv, np.float32)
    ln2_scale = np.asarray(ln2_scale, np.float32)
    ln2_shift = np.asarray(ln2_shift, np.float32)
    w_fc = np.asarray(w_fc, np.float32)
    b_fc = np.asarray(b_fc, np.float32)
    w_proj = np.asarray(w_proj, np.float32)
    b_proj = np.asarray(b_proj, np.float32)

    # fold LN1 affine into qkv weights
    w1 = ln1_scale[:, None] * w_qkv  # [D, 3D]
    b1 = b_qkv + ln1_shift @ w_qkv  # [3D]
    sc = 1.0 / np.sqrt(HD)
    wq = w1[:, :D] * sc
    bq = b1[:D] * sc
    wk = w1[:, D : 2 * D]
    bk = b1[D : 2 * D]
    wv = w1[:, 2 * D :]
    bv = b1[2 * D :]

    wqp_h = (_pretile(wq, D_T, D_T) * 256.0).astype(fp8)
    wkp_h = (_pretile(wk, D_T, D_T) * 256.0).astype(fp8)
    bqk_h = np.ascontiguousarray(
        np.concatenate([bq, bk]).reshape(2 * D_T, P).T
    )  # [128, 16] f32

    wva_h = np.zeros((D, VA), np.float32)
    bva_h = np.zeros((1, VA), np.float32)
    for h in range(H):
        wva_h[:, h * HDA : h * HDA + HD] = wv[:, h * HD : (h + 1) * HD]
        bva_h[0, h * HDA : h * HDA + HD] = bv[h * HD : (h + 1) * HD]
        bva_h[0, h * HDA + HD] = 1.0  # denominator ones column
    wva_h = wva_h.astype(bf16)
    bva_h = bva_h.astype(bf16)

    # fold LN2 affine into fc; pre-tile bf16
    wfc_f = ln2_scale[:, None] * w_fc
    wfcp_h = _pretile(wfc_f, M_T, D_T).astype(bf16)  # [4096, 1024]
    wprojp_h = _pretile(w_proj, D_T, M_T).astype(bf16)  # [1024, 4096]
    bfc_h = np.ascontiguousarray((b_fc + ln2_shift @ w_fc).reshape(M_T, P).T)
    bproj_h = np.ascontiguousarray(b_proj.reshape(D_T, P).T)  # [128, 8]

    def put(buf, off, arr):
        bts = np.ascontiguousarray(arr).view(np.uint8).reshape(-1)
        buf[off : off + bts.size] = bts

    base = np.zeros(NB, np.uint8)
    put(base, OFF_WQ, wqp_h)
    put(base, OFF_WK, wkp_h)
    put(base, OFF_WVA, wva_h)
    put(base, OFF_WFC, wfcp_h)
    put(base, OFF_WPJ, wprojp_h)
    put(base, OFF_BQK, bqk_h)
    put(base, OFF_BFC, bfc_h)
    put(base, OFF_BPJ, bproj_h)
    put(base, OFF_BVA, bva_h)

    # ragged diagonal masks: key index i (partition) vs query index col:
    # keep if i <= col (key class c <= own class j) else i < col.
    ii = np.arange(P)[:, None]
    qq = np.arange(P)[None, :]
    tri_inc = (ii <= qq).astype(np.float32)  # [128,128]
    tri_exc = (ii < qq).astype(np.float32)

    in_maps = []
    for core in range(N_CORES):
        b, j = divmod(core, CLS)
        # class-major context, own class last: block bi holds class (j+1+bi)%4
        blocks = [(j + 1 + bi) % CLS for bi in range(CLS)]
        xperm = np.concatenate([x[b, c::CLS] for c in blocks], axis=0)  # [2048, D]
        tri_h = np.empty((P, CLS, 2, P), np.float32)
        for bi, c in enumerate(blocks):
            m = tri_inc if c <= j else tri_exc
            tri_h[:, bi, 0, :] = m
            tri_h[:, bi, 1, :] = m
        pkc = base.copy()
        put(pkc, OFF_X, np.ascontiguousarray(xperm.astype(bf16)))
        put(pkc, OFF_TRI, tri_h.astype(fp8))
        in_maps.append({"pk": pkc.reshape(1, NB)})
    return in_maps


def assemble_output(results):
    out = np.empty((B, T, D), np.float32)
    for core in range(N_CORES):
        b, j = divmod(core, CLS)
        out[b, j::CLS, :] = np.asarray(results[core]["out"], np.float32)
    return out


def kernel(**inputs) -> np.ndarray:
    from concourse.bass_utils import run_bass_kernel_spmd

    in_maps = make_in_maps(**inputs)
    bva = np.frombuffer(
        in_maps[0]["pk"][0, OFF_BVA : OFF_BVA + VA * 2].tobytes(),
        dtype=mybir.dt.np(BF16),
    ).astype(np.float32)
    mask = np.ones(VA, bool)
    mask[HD::HDA] = False  # the ones columns
    nc = _get_program(bv_nonzero=bool(np.any(bva[mask] != 0.0)))
    res = run_bass_kernel_spmd(nc, in_maps, core_ids=list(range(N_CORES)))
    return assemble_output(res.results)


# revision 29
# speedup vs baseline: 3.0711x; 1.0948x over previous
"""Trainium2 Bass kernel for a dense transformer block (LN->causal attn->res->LN->MLP->res).

Shapes (hardcoded): x [2, 2048, 1024], 16 heads, head_dim 64, MLP hidden 4096, fp32 out.

v6 sharding: 8 cores = (batch b in {0,1}) x (token class j in {0..3}).
Class j = tokens {t : t % 4 == j} (512 tokens, position order).  Each core
gets the full 2048-token context CLASS-MAJOR with its own class LAST
(block order (j+1)%4, (j+2)%4, (j+3)%4, j), computes LN1 + K/V over the whole
context and Q for its own class, then runs causal-skip attention: any class
block's 128-token tile t covers positions [512t, 512(t+1)), so key tile
(block, t) is needed only by query tiles q >= t -- tiles with t > q are never
computed (37.5% of score/exp/AV work skipped, uniformly on every core; the
interleave balances the causal triangle).  The ragged diagonal (q == t) is a
per-core 0/1 mask (tri if the block's class <= own class else strictly-lower)
multiplied after the exp.  LN2/MLP/residuals for the own 512 tokens; the host
scatters rows back to positions j::4.  (A K/V AllGather variant was tried:
the 4-core gather costs ~120us/collective on this runtime and loses to the
~80us of duplicated K/V compute it saves.)

P1/P2 are fused into one batch pipeline: per 4-tile x batch, LN -> (Q for the
own batch) -> the K context chunk and V tiles that batch enables, so the PE
never waits for the whole LN pass (engines execute in program order; the
earlier phase-ordered version left the PE stalled behind not-yet-ready
transposes).  DMA issue order is tuned: batch-0 x goes out first (WVA after
it, trimask just before attention) so LN starts ~10us earlier, and the output
writes back in two 1MB halves as their columns finalize.

Carried over from v3/v4: ONE packed uint8 input tensor (runtime staging costs
~63us per tensor + ~10us/MB per exec, dominating the wall clock); wq/wk
shipped fp8e4m3 x256 (their quantization only perturbs pre-softmax scores:
+~4e-3 rel err, fine) with 1/256 folded into the Q/K epilogues; wv/wfc/wproj
stay bf16 (fp8 there costs ~1e-2 each and fails the 2e-2 gate); bf16-only x;
V augmented with a per-head ones column accumulating the softmax denominator.
Output ships bf16 (half-ulp <=2e-3 on |out|max 6.6; host upcasts) to halve
output staging.
"""

from contextlib import ExitStack

import numpy as np

import concourse.bacc as bacc
import concourse.mybir as mybir
import concourse.tile as tile
from concourse.masks import make_identity

F32 = mybir.dt.float32
BF16 = mybir.dt.bfloat16
FP8 = mybir.dt.float8e4
AF = mybir.ActivationFunctionType
ALU = mybir.AluOpType

B = 2
T = 2048
D = 1024
H = 16
HD = 64
HDA = HD + 1  # +1 denominator column per head
MLP = 4096
NQ = 512  # tokens per core
CTX = T
EPS = 1e-5

N_CORES = 8
P = 128
CLS = 4

D_T = D // P  # 8
Q_T = NQ // P  # 4 query tiles (also key tiles per class)
M_T = MLP // P  # 32
VA = H * HDA  # 1040 augmented V width

REPLICA_GROUPS = [[0, 1, 2, 3], [4, 5, 6, 7]]

# ---- packed-input layout (bytes). All segments 4KB-aligned. ----
def _align(x, a=4096):
    return (x + a - 1) // a * a


_off = 0
def _seg(nbytes):
    global _off
    o = _off
    _off = _align(_off + nbytes)
    return o


OFF_X = _seg(CTX * D * 2)           # bf16 [2048,1024] class-major, own class last
OFF_WQ = _seg(D * D)                # fp8 [1024,1024] pretiled (x256)
OFF_WK = _seg(D * D)                # fp8 [1024,1024] pretiled (x256)
OFF_WVA = _seg(D * VA * 2)          # bf16 [1024,1040]
OFF_WFC = _seg(MLP * D * 2)         # bf16 [4096,1024] pretiled
OFF_WPJ = _seg(D * MLP * 2)         # bf16 [1024,4096] pretiled
OFF_TRI = _seg(P * CLS * 2 * P)     # fp8 0/1 [128, 4, 2, 128]
OFF_BQK = _seg(P * 2 * D_T * 4)     # f32 [128,16]
OFF_BFC = _seg(P * M_T * 4)         # f32 [128,32]
OFF_BPJ = _seg(P * D_T * 4)         # f32 [128,8]
OFF_BVA = _seg(VA * 2)              # bf16 [1,1040]
NB = _align(_off)


def build_program(loop_n: int = 1, bv_nonzero: bool = False):
    """Emit the SPMD Bass program. Returns finalized nc."""
    nc = bacc.Bacc("TRN2", target_bir_lowering=False)

    pk = nc.dram_tensor("pk", [1, NB], mybir.dt.uint8, kind="ExternalInput")
    out = nc.dram_tensor("out", [NQ, D], BF16, kind="ExternalOutput")

    def view(off, nbytes, dt):
        return pk[0, off : off + nbytes].bitcast(dt)

    with tile.TileContext(nc) as tc:
        with ExitStack() as ctx:
            if loop_n > 1:
                ctx.enter_context(tc.For_i(0, loop_n, 1))
            const = ctx.enter_context(tc.tile_pool(name="const", bufs=1))
            identity = const.tile([P, P], F32)
            make_identity(nc, identity)
            identity_bf = const.tile([P, P], BF16)
            make_identity(nc, identity_bf)
            ones1 = const.tile([1, P], BF16)
            nc.vector.memset(ones1, 1.0)
            eps_t = const.tile([P, 1], F32)
            nc.vector.memset(eps_t, EPS)
            bqk_sb = const.tile([P, 2 * D_T], F32)
            nc.sync.dma_start(
                bqk_sb, view(OFF_BQK, P * 2 * D_T * 4, F32).rearrange("(p c) -> p c", p=P)
            )
            bva_sb = const.tile([1, VA], BF16)
            nc.sync.dma_start(
                bva_sb, view(OFF_BVA, VA * 2, BF16).rearrange("(p c) -> p c", p=1)
            )
            tri8 = const.tile([P, CLS, 2, P], FP8)
            tri_sb = const.tile([P, CLS, 2, P], BF16)

            # Long-lived pools.
            qt_cm = tc.tile_pool(name="qt", bufs=1)
            qt_pool = qt_cm.__enter__()
            QT = [qt_pool.tile([P, NQ], BF16, name=f"QT{i}") for i in range(D_T)]
            kt_cm = tc.tile_pool(name="ktp", bufs=1)
            kt_pool = kt_cm.__enter__()
            KTT = kt_pool.tile([P, D_T, CTX], BF16, name="KTT")
            vsb_cm = tc.tile_pool(name="vsb", bufs=1)
            vsb_pool = vsb_cm.__enter__()
            VSBT = vsb_pool.tile([P, CLS * Q_T, VA], BF16, name="VSBT")

            # RIGHT pools (live into P4/P5)
            x2_pool = ctx.enter_context(tc.tile_pool(name="x2", bufs=1, side="right"))
            X2 = [x2_pool.tile([P, D], F32, name=f"X2{i}") for i in range(Q_T)]
            l2t_pool = ctx.enter_context(
                tc.tile_pool(name="l2t", bufs=1, side="right")
            )
            L2T = [l2t_pool.tile([P, NQ], BF16, name=f"L2T{i}") for i in range(D_T)]
            xo_pool = ctx.enter_context(tc.tile_pool(name="xo", bufs=1, side="right"))
            XO = xo_pool.tile([P, Q_T, D], BF16, name="XO")
            # xnT on top of the RIGHT stack; freed after Q/K/V, wfc prefetch
            # reuses the space during attention.
            xnt_cm = tc.tile_pool(name="xnt", bufs=1, side="right")
            xnt_pool = xnt_cm.__enter__()
            xnT = [xnt_pool.tile([P, CTX], BF16, name=f"xnT{i}") for i in range(D_T)]

            # -------- P1+P2 fused: per x-batch LN -> (Q) -> K chunk -> V tiles
            # PE stays fed: batch 0 (own class, ctx cols 1536:2048) lands
            # first, then Q, then each later batch's LN overlaps the previous
            # batch's K/V matmuls.  wk streams per (nt, mt); WVA is resident.
            xcb_v = view(OFF_X, CTX * D * 2, BF16)
            batches = ((12, 13, 14, 15), (0, 1, 2, 3), (4, 5, 6, 7), (8, 9, 10, 11))
            ntof = (3, 0, 1, 2)  # K context chunk produced after each batch
            with tc.tile_pool(name="p2vw", bufs=1) as p2vw, tc.tile_pool(
                name="p1work", bufs=2
            ) as p1w, tc.tile_pool(name="p1xn", bufs=4) as p1xn, tc.tile_pool(
                name="p1stat", bufs=6
            ) as p1s, tc.tile_pool(name="p2w", bufs=2) as p2w, tc.tile_pool(
                name="p1ps", bufs=4, space="PSUM"
            ) as p1ps, tc.tile_pool(
                name="p2ps", bufs=2, space="PSUM"
            ) as p2ps, tc.tile_pool(
                name="p2vps", bufs=2, space="PSUM", side="right"
            ) as p2vps:
                WVA = p2vw.tile([P, D_T, VA], BF16, name="wva")
                vchunks = [(0, 512), (512, 512), (1024, VA - 1024)]
                for bi, bt in enumerate(batches):
                    if bi == 0:
                        xt = XO
                    else:
                        xt = p1w.tile([P, 4, D], BF16, tag="xt")
                    nc.sync.dma_start(
                        xt,
                        xcb_v[bt[0] * P * D : (bt[0] + 4) * P * D].rearrange(
                            "(a p c) -> p a c", p=P, c=D
                        ),
                    )
                    if bi == 0:
                        nc.sync.dma_start(
                            WVA,
                            view(OFF_WVA, D * VA * 2, BF16).rearrange(
                                "(a p c) -> p a c", p=P, c=VA
                            ),
                        )
                    xns = []
                    for ai, tt in enumerate(bt):
                        stats = p1s.tile([P, 2, 6], F32, tag="stats")
                        for g in range(2):
                            nc.vector.bn_stats(
                                stats[:, g, :], xt[:, ai, g * 512 : (g + 1) * 512]
                            )
                        mv = p1s.tile([P, 2], F32, tag="mv")
                        nc.vector.bn_aggr(mv, stats)
                        sd = p1s.tile([P, 1], F32, tag="sd")
                        nc.scalar.activation(sd, mv[:, 1:2], AF.Sqrt, bias=eps_t)
                        rstd = p1s.tile([P, 1], F32, tag="rstd")
                        nc.vector.reciprocal(rstd, sd)
                        nmb = p1s.tile([P, 1], F32, tag="nmb")
                        nc.vector.tensor_scalar(
                            nmb, mv[:, 0:1], rstd, -1.0, ALU.mult, ALU.mult
                        )
                        xn = p1xn.tile([P, D], BF16, tag="xn")
                        nc.scalar.activation(
                            xn, xt[:, ai, :], AF.Identity, bias=nmb, scale=rstd
                        )
                        xns.append(xn)
                    for dt_ in range(D_T):
                        tp = p1ps.tile([P, 4, P], BF16, tag="tp")
                        for ai in range(4):
                            nc.tensor.transpose(
                                tp[:, ai, :],
                                xns[ai][:, dt_ * P : (dt_ + 1) * P],
                                identity_bf,
                            )
                        if dt_ % 2 == 0:
                            nc.vector.tensor_copy(
                                xnT[dt_][:, bt[0] * P : (bt[0] + 4) * P], tp
                            )
                        else:
                            nc.scalar.copy(
                                xnT[dt_][:, bt[0] * P : (bt[0] + 4) * P], tp
                            )
                    nt = ntof[bi]
                    if bi == 0:
                        # Q^T for the own class (ctx cols 1536:2048)
                        for mt in range(D_T):
                            ws = p2w.tile([P, D_T, P], FP8, tag="wsk")
                            nc.sync.dma_start(
                                ws,
                                view(
                                    OFF_WQ + mt * P * D, P * D, FP8
                                ).rearrange("(p a c) -> p a c", p=P, c=P),
                            )
                            ps = p2ps.tile([P, NQ], F32, tag="ps")
                            for kt_ in range(D_T):
                                nc.tensor.matmul(
                                    ps,
                                    ws[:, kt_, :],
                                    xnT[kt_][:, CTX - NQ :],
                                    start=(kt_ == 0),
                                    stop=(kt_ == D_T - 1),
                                )
                            nc.scalar.activation(
                                QT[mt],
                                ps,
                                AF.Identity,
                                bias=bqk_sb[:, mt : mt + 1],
                                scale=1.0 / 256.0,
                            )
                    # K^T chunk nt (ctx cols nt*512 .. +512)
                    for mt in range(D_T):
                        ws = p2w.tile([P, D_T, P], FP8, tag="wsk")
                        nc.sync.dma_start(
                            ws,
                            view(OFF_WK + mt * P * D, P * D, FP8).rearrange(
                                "(p a c) -> p a c", p=P, c=P
                            ),
                        )
                        ps = p2ps.tile([P, 512], F32, tag="ps")
                        for kt_ in range(D_T):
                            nc.tensor.matmul(
                                ps,
                                ws[:, kt_, :],
                                xnT[kt_][:, nt * 512 : (nt + 1) * 512],
                                start=(kt_ == 0),
                                stop=(kt_ == D_T - 1),
                            )
                        nc.vector.tensor_scalar(
                            KTT[:, mt, nt * 512 : (nt + 1) * 512],
                            ps,
                            1.0 / 256.0,
                            bqk_sb[:, D_T + mt : D_T + mt + 1],
                            ALU.mult,
                            ALU.add,
                        )
                    # V_aug for this batch's 4 context tiles
                    for mtv in bt:
                        for ci, (c0, cw) in enumerate(vchunks):
                            ps = p2vps.tile([P, 512], F32, tag="ps")
                            for kt_ in range(D_T):
                                nc.tensor.matmul(
                                    ps[:, :cw],
                                    xnT[kt_][:, mtv * P : (mtv + 1) * P],
                                    WVA[:, kt_, c0 : c0 + cw],
                                    start=(kt_ == 0),
                                    stop=(kt_ == D_T - 1 and not bv_nonzero),
                                )
                            if bv_nonzero:
                                nc.tensor.matmul(
                                    ps[:, :cw],
                                    ones1,
                                    bva_sb[:, c0 : c0 + cw],
                                    start=False,
                                    stop=True,
                                )
                            if ci % 2 == 0:
                                nc.vector.tensor_copy(
                                    VSBT[:, mtv, c0 : c0 + cw], ps[:, :cw]
                                )
                            else:
                                nc.scalar.copy(
                                    VSBT[:, mtv, c0 : c0 + cw], ps[:, :cw]
                                )
                if not bv_nonzero:
                    ones_cols = VSBT.rearrange("p t (h c) -> p t h c", c=HDA)[
                        :, :, :, HD : HD + 1
                    ]
                    nc.vector.memset(ones_cols, 1.0)

            # xnT consumed -> free; prefetch half of wfc during attention.
            xnt_cm.__exit__(None, None, None)
            MT_RES = M_T // 2
            wfc_cm = tc.tile_pool(name="wfcp_sb", bufs=1, side="right")
            wfc_pool = wfc_cm.__enter__()
            WFC = wfc_pool.tile([P, MT_RES, D_T, P], BF16, name="WFC")
            nc.sync.dma_start(
                WFC,
                view(OFF_WFC, MT_RES * P * D * 2, BF16).rearrange(
                    "(a p c) -> p a c", p=P, c=D
                ).rearrange("p a (k c) -> p a k c", c=P),
            )

            # -------- P3: causal-skip attention, key tile (c, t) -------------
            # key tile (class c, tile t) serves query tiles q in [t, 4); the
            # first 128 query columns (q == t) get the ragged tri mask.
            nc.sync.dma_start(
                tri8,
                view(OFF_TRI, P * CLS * 2 * P, FP8).rearrange(
                    "(p c s q) -> p c s q", p=P, c=CLS, s=2
                ),
            )
            nc.vector.tensor_copy(tri_sb, tri8)
            ptp_cm = tc.tile_pool(name="ptp", bufs=4)
            ptp = ptp_cm.__enter__()
            p3s_cm = tc.tile_pool(name="p3s", bufs=2)
            p3s = p3s_cm.__enter__()
            stps_cm = tc.tile_pool(name="stps", bufs=2, space="PSUM")
            stps = stps_cm.__enter__()
            yps_cm = tc.tile_pool(name="yps", bufs=2, space="PSUM")
            yps = yps_cm.__enter__()
            for hp in range(H // 2):
                yp = yps.tile([P, 2, Q_T, P], F32, name=f"yp{hp}", tag="yp")
                for t in range(Q_T):
                    nqc = (Q_T - t) * P  # query columns t*128 .. 512
                    for c in range(CLS):
                        kti = c * Q_T + t
                        kcol = c * NQ + t * P
                        # fixed 512-wide halves keep each matmul's PSUM
                        # region inside one 2KB bank
                        st = stps.tile([P, 2, NQ], F32, tag="st")
                        for s in range(2):
                            nc.tensor.matmul(
                                st[:, s, :nqc],
                                KTT[s * HD : (s + 1) * HD, hp, kcol : kcol + P],
                                QT[hp][s * HD : (s + 1) * HD, t * P :],
                                start=True,
                                stop=True,
                                tile_position=(s * HD, 0),
                            )
                        pt = ptp.tile([P, 2, nqc], BF16, tag="pt")
                        nc.scalar.activation(pt, st[:, :, :nqc], AF.Exp)
                        nc.vector.tensor_mul(
                            pt[:, :, 0:P], pt[:, :, 0:P], tri_sb[:, c, :, :]
                        )
                        # FLIPPED AV: stationary = pt [128 keys x 128 queries]
                        # (fills the PE array), moving = V_aug [128, 65] ->
                        # out [128 queries, 65] per (s, q); 65-cycle streams.
                        # Bank per s; q==0's start=True zeroes it, one stop at
                        # the bank's last writer (t=3,c=3,q=3).
                        for s in range(2):
                            h = 2 * hp + s
                            for q in range(t, Q_T):
                                nc.tensor.matmul(
                                    yp[:, s, q, :HDA],
                                    pt[:, s, (q - t) * P : (q - t + 1) * P],
                                    VSBT[:, kti, h * HDA : (h + 1) * HDA],
                                    start=(t == 0 and c == 0 and q == 0),
                                    stop=(
                                        t == Q_T - 1
                                        and c == CLS - 1
                                        and q == Q_T - 1
                                    ),
                                )
                for s in range(2):
                    h2 = 2 * hp + s
                    for q in range(Q_T):
                        recip = p3s.tile([P, 1], F32, tag="recip")
                        nc.vector.reciprocal(recip, yp[:, s, q, HD : HD + 1])
                        ynorm = p3s.tile([P, HD], F32, tag="ynorm")
                        nc.vector.tensor_scalar_mul(ynorm, yp[:, s, q, :HD], recip)
                        nc.vector.tensor_add(
                            X2[q][:, h2 * HD : (h2 + 1) * HD],
                            XO[:, q, h2 * HD : (h2 + 1) * HD],
                            ynorm,
                        )

            yps_cm.__exit__(None, None, None)
            stps_cm.__exit__(None, None, None)
            p3s_cm.__exit__(None, None, None)
            ptp_cm.__exit__(None, None, None)
            vsb_cm.__exit__(None, None, None)
            kt_cm.__exit__(None, None, None)
            qt_cm.__exit__(None, None, None)

            # ---------------- P4: residual + LN2 + transpose -----------------
            with tc.tile_pool(name="p4w", bufs=3) as p4w, tc.tile_pool(
                name="p4s", bufs=4
            ) as p4s, tc.tile_pool(name="p4ps", bufs=4, space="PSUM") as p4ps:
                for tt in range(Q_T):
                    stats = p4s.tile([P, 2, 6], F32, tag="stats2")
                    for g in range(2):
                        nc.vector.bn_stats(
                            stats[:, g, :], X2[tt][:, g * 512 : (g + 1) * 512]
                        )
                    mv = p4s.tile([P, 2], F32, tag="mv2")
                    nc.vector.bn_aggr(mv, stats)
                    sd = p4s.tile([P, 1], F32, tag="sd2")
                    nc.scalar.activation(sd, mv[:, 1:2], AF.Sqrt, bias=eps_t)
                    rstd = p4s.tile([P, 1], F32, tag="rstd2")
                    nc.vector.reciprocal(rstd, sd)
                    nmb = p4s.tile([P, 1], F32, tag="nmb2")
                    nc.vector.tensor_scalar(
                        nmb, mv[:, 0:1], rstd, -1.0, ALU.mult, ALU.mult
                    )
                    l2 = p4w.tile([P, D], BF16, tag="l2")
                    nc.scalar.activation(l2, X2[tt], AF.Identity, bias=nmb, scale=rstd)
                    for mt in range(D_T):
                        tp = p4ps.tile([P, P], BF16, tag="tpb")
                        nc.tensor.transpose(
                            tp, l2[:, mt * P : (mt + 1) * P], identity_bf
                        )
                        if mt % 2 == 0:
                            nc.vector.tensor_copy(
                                L2T[mt][:, tt * P : (tt + 1) * P], tp
                            )
                        else:
                            nc.scalar.copy(L2T[mt][:, tt * P : (tt + 1) * P], tp)

            # ---------------- P5: MLP + final residual ----------------
            with tc.tile_pool(name="h1t", bufs=1) as h1t_pool, tc.tile_pool(
                name="p5w", bufs=2
            ) as p5w, tc.tile_pool(name="p5o", bufs=1) as p5o, tc.tile_pool(
                name="p5ps", bufs=3, space="PSUM"
            ) as p5ps, tc.tile_pool(
                name="p5tps", bufs=4, space="PSUM"
            ) as p5tps:
                bfc_sb = p5o.tile([P, M_T], F32)
                nc.sync.dma_start(
                    bfc_sb,
                    view(OFF_BFC, P * M_T * 4, F32).rearrange("(p c) -> p c", p=P),
                )
                bproj_sb = p5o.tile([P, D_T], F32)
                nc.sync.dma_start(
                    bproj_sb,
                    view(OFF_BPJ, P * D_T * 4, F32).rearrange("(p c) -> p c", p=P),
                )
                OUT = p5o.tile([P, Q_T, D], BF16, name="OUT")
                H1T = [h1t_pool.tile([P, NQ], BF16, name=f"H1T{i}") for i in range(M_T)]
                for mt in range(M_T):
                    if mt < MT_RES:
                        wfc_t = WFC[:, mt]
                    else:
                        wfc_t = p5w.tile([P, D_T, P], BF16, tag="wsf")
                        nc.sync.dma_start(
                            wfc_t,
                            view(OFF_WFC + mt * P * D * 2, P * D * 2, BF16).rearrange(
                                "(p k c) -> p k c", p=P, c=P
                            ),
                        )
                    ps = p5ps.tile([P, NQ], F32, tag="ps")
                    for kt_ in range(D_T):
                        nc.tensor.matmul(
                            ps,
                            wfc_t[:, kt_, :],
                            L2T[kt_],
                            start=(kt_ == 0),
                            stop=(kt_ == D_T - 1),
                        )
                    nc.scalar.activation(
                        H1T[mt], ps, AF.Relu, bias=bfc_sb[:, mt : mt + 1]
                    )
                wfc_cm.__exit__(None, None, None)
                for mt in range(D_T):
                    ws = p5w.tile([P, M_T, P], BF16, tag="wsp")
                    nc.sync.dma_start(
                        ws,
                        view(OFF_WPJ + mt * P * MLP * 2, P * MLP * 2, BF16).rearrange(
                            "(p a c) -> p a c", p=P, c=P
                        ),
                    )
                    ps = p5ps.tile([P, NQ], F32, tag="ps")
                    for kt_ in range(M_T):
                        nc.tensor.matmul(
                            ps,
                            ws[:, kt_, :],
                            H1T[kt_],
                            start=(kt_ == 0),
                            stop=(kt_ == M_T - 1),
                        )
                    mlpt = p5w.tile([P, NQ], F32, tag="mlpt")
                    nc.vector.tensor_scalar_add(mlpt, ps, bproj_sb[:, mt : mt + 1])
                    for tt in range(Q_T):
                        tp = p5tps.tile([P, P], F32, tag="tp")
                        nc.tensor.transpose(
                            tp, mlpt[:, tt * P : (tt + 1) * P], identity
                        )
                        nc.vector.tensor_add(
                            OUT[:, tt, mt * P : (mt + 1) * P],
                            X2[tt][:, mt * P : (mt + 1) * P],
                            tp,
                        )
                    if mt == D_T // 2 - 1 or mt == D_T - 1:
                        h0 = 0 if mt < D_T // 2 else D // 2
                        nc.sync.dma_start(
                            out.rearrange("(a p) c -> p a c", p=P)[
                                :, :, h0 : h0 + D // 2
                            ],
                            OUT[:, :, h0 : h0 + D // 2],
                        )


    nc.finalize()
    return nc


_PROG = {}


def _get_program(bv_nonzero: bool = False):
    if bv_nonzero not in _PROG:
        _PROG[bv_nonzero] = build_program(bv_nonzero=bv_nonzero)
    return _PROG[bv_nonzero]


def _pretile(w, n_out_tiles, n_k_tiles):
    """[K, N] -> lhsT pre-tiled layout: row (mt*128+p), flat col (kt*128+c)
    holds w[kt*128 + p, mt*128 + c]."""
    K, N = w.shape
    assert K == n_k_tiles * P and N == n_out_tiles * P
    # axes (kt, p, mt, c) -> (mt, p, kt, c)
    return np.ascontiguousarray(
        w.reshape(n_k_tiles, P, n_out_tiles, P)
        .transpose(2, 1, 0, 3)
        .reshape(n_out_tiles * P, n_k_tiles * P)
    )


def make_in_maps(x, ln1_scale, ln1_shift, w_qkv, b_qkv, ln2_scale, ln2_shift,
                 w_fc, b_fc, w_proj, b_proj):
    """Host-side prep: fold LN affine into weights, prescale Q by 1/sqrt(hd),
    augment V with the ones column, pre-tile weights (bf16), build the
    per-core class-interleaved x slice + ragged-diagonal masks, and pack
    everything into one uint8 tensor per core."""
    import ml_dtypes

    bf16 = ml_dtypes.bfloat16
    fp8 = mybir.dt.np(FP8)

    x = np.asarray(x, np.float32)
    ln1_scale = np.asarray(ln1_scale, np.float32)
    ln1_shift = np.asarray(ln1_shift, np.float32)
    w_qkv = np.asarray(w_qkv, np.float32)
    b_qkv = np.asarray(b_qkv, np.float32)
    ln2_scale = np.asarray(ln2_scale, np.float32)
    ln2_shift = np.asarray(ln2_shift, np.float32)
    w_fc = np.asarray(w_fc, np.float32)
    b_fc = np.asarray(b_fc, np.float32)
    w_proj = np.asarray(w_proj, np.float32)
    b_proj = np.asarray(b_proj, np.float32)

    # fold LN1 affine into qkv weights
    w1 = ln1_scale[:, None] * w_qkv  # [D, 3D]
    b1 = b_qkv + ln1_shift @ w_qkv  # [3D]
    sc = 1.0 / np.sqrt(HD)
    wq = w1[:, :D] * sc
    bq = b1[:D] * sc
    wk = w1[:, D : 2 * D]
    bk = b1[D : 2 * D]
    wv = w1[:, 2 * D :]
    bv = b1[2 * D :]

    wqp_h = (_pretile(wq, D_T, D_T) * 256.0).astype(fp8)
    wkp_h = (_pretile(wk, D_T, D_T) * 256.0).astype(fp8)
    bqk_h = np.ascontiguousarray(
        np.concatenate([bq, bk]).reshape(2 * D_T, P).T
    )  # [128, 16] f32

    wva_h = np.zeros((D, VA), np.float32)
    bva_h = np.zeros((1, VA), np.float32)
    for h in range(H):
        wva_h[:, h * HDA : h * HDA + HD] = wv[:, h * HD : (h + 1) * HD]
        bva_h[0, h * HDA : h * HDA + HD] = bv[h * HD : (h + 1) * HD]
        bva_h[0, h * HDA + HD] = 1.0  # denominator ones column
    wva_h = wva_h.astype(bf16)
    bva_h = bva_h.astype(bf16)

    # fold LN2 affine into fc; pre-tile bf16
    wfc_f = ln2_scale[:, None] * w_fc
    wfcp_h = _pretile(wfc_f, M_T, D_T).astype(bf16)  # [4096, 1024]
    wprojp_h = _pretile(w_proj, D_T, M_T).astype(bf16)  # [1024, 4096]
    bfc_h = np.ascontiguousarray((b_fc + ln2_shift @ w_fc).reshape(M_T, P).T)
    bproj_h = np.ascontiguousarray(b_proj.reshape(D_T, P).T)  # [128, 8]

    def put(buf, off, arr):
        bts = np.ascontiguousarray(arr).view(np.uint8).reshape(-1)
        buf[off : off + bts.size] = bts

    base = np.zeros(NB, np.uint8)
    put(base, OFF_WQ, wqp_h)
    put(base, OFF_WK, wkp_h)
    put(base, OFF_WVA, wva_h)
    put(base, OFF_WFC, wfcp_h)
    put(base, OFF_WPJ, wprojp_h)
    put(base, OFF_BQK, bqk_h)
    put(base, OFF_BFC, bfc_h)
    put(base, OFF_BPJ, bproj_h)
    put(base, OFF_BVA, bva_h)

    # ragged diagonal masks: key index i (partition) vs query index col:
    # keep if i <= col (key class c <= own class j) else i < col.
    ii = np.arange(P)[:, None]
    qq = np.arange(P)[None, :]
    tri_inc = (ii <= qq).astype(np.float32)  # [128,128]
    tri_exc = (ii < qq).astype(np.float32)

    in_maps = []
    for core in range(N_CORES):
        b, j = divmod(core, CLS)
        # class-major context, own class last: block bi holds class (j+1+bi)%4
        blocks = [(j + 1 + bi) % CLS for bi in range(CLS)]
        xperm = np.concatenate([x[b, c::CLS] for c in blocks], axis=0)  # [2048, D]
        tri_h = np.empty((P, CLS, 2, P), np.float32)
        for bi, c in enumerate(blocks):
            m = tri_inc if c <= j else tri_exc
            tri_h[:, bi, 0, :] = m
            tri_h[:, bi, 1, :] = m
        pkc = base.copy()
        put(pkc, OFF_X, np.ascontiguousarray(xperm.astype(bf16)))
        put(pkc, OFF_TRI, tri_h.astype(fp8))
        in_maps.append({"pk": pkc.reshape(1, NB)})
    return in_maps


def assemble_output(results):
    out = np.empty((B, T, D), np.float32)
    for core in range(N_CORES):
        b, j = divmod(core, CLS)
        out[b, j::CLS, :] = np.asarray(results[core]["out"], np.float32)
    return out


def kernel(**inputs) -> np.ndarray:
    from concourse.bass_utils import run_bass_kernel_spmd

    in_maps = make_in_maps(**inputs)
    bva = np.frombuffer(
        in_maps[0]["pk"][0, OFF_BVA : OFF_BVA + VA * 2].tobytes(),
        dtype=mybir.dt.np(BF16),
    ).astype(np.float32)
    mask = np.ones(VA, bool)
    mask[HD::HDA] = False  # the ones columns
    nc = _get_program(bv_nonzero=bool(np.any(bva[mask] != 0.0)))
    res = run_bass_kernel_spmd(nc, in_maps, core_ids=list(range(N_CORES)))
    return assemble_output(res.results)
